# revision 1
# baseline (speedup 1.0000x reference)
"""Trainium2 Bass kernel for a Mixtral decoder layer (attention + top-2 MoE).

Strategy (8 NeuronCores):
  Launch 1 (attention): 2D shard = (batch b in {0,1}) x (head-group g in {0..3},
    4 heads / 256 feature slice each). Each core computes q/k/v projections for
    its slice, transposed-scores flash-style attention (scores computed as
    s^T[tk, tq] so the softmax denominator folds into a ones-column of V), and
    a partial output projection. Host sums the 4 partials per batch.
  Host: residual add, rmsnorm, gating logits, exact top-2 routing, per-expert
    token gather (expert-parallel dispatch done in numpy - free).
  Launch 2 (MoE FFN): expert-parallel - core e owns expert e's w1/w3/w2 and
    processes its routed tokens (padded to capacity C) densely, pipelined over
    512-token blocks.
  Host: scatter-add expert outputs + residual. All matmuls bf16 with fp32 PSUM
    accumulation; softmax/normalization/routing in fp32.
"""
import os
import sys

import numpy as np
import ml_dtypes

for _p in ("/root/.axon_site", "/root/.axon_site/_ro/trn_rl_repo", "/opt/trn_rl_repo"):
    if os.path.isdir(_p) and _p not in sys.path:
        sys.path.append(_p)

import concourse.tile as tile
from concourse import bacc, mybir
from concourse.bass_utils import run_bass_kernel_spmd

BF16 = ml_dtypes.bfloat16
AF = mybir.ActivationFunctionType
ALU = mybir.AluOpType
DT = mybir.dt

H = 1024
S = 2048
B = 2
NH = 16
D = 64
E = 8
I = 2048
T = B * S
EPS = 1e-5

NCORES = 8
NGRP = 4              # head groups (cores per batch)
NHPC = NH // NGRP     # 4 heads per core
DS = NHPC * D         # 256-wide feature slice per core
TQC = 4               # tq chunks of 512
NTK = S // 128        # 16 tk tiles
NCI = H // 128        # 8 contraction chunks

C = 1152              # MoE expert token capacity (per-expert max on this data ~1087)

_CACHE = {}
LAST_RESULTS = []     # BassKernelResults of the last kernel() call (for test harness)
TRACE = os.environ.get("KERNEL_TRACE", "0") == "1"


def _capacity_chunks(cap):
    out, o = [], 0
    while o < cap:
        ln = min(512, cap - o)
        out.append((o, ln))
        o += ln
    return out


def _build_l1():
    nc = bacc.Bacc("TRN2", target_bir_lowering=False, debug=False, num_devices=NCORES)
    xT = nc.dram_tensor("xT", [H, S], DT.bfloat16, kind="ExternalInput")
    wqT = nc.dram_tensor("wqT", [H, DS], DT.bfloat16, kind="ExternalInput")
    wkT = nc.dram_tensor("wkT", [H, DS], DT.bfloat16, kind="ExternalInput")
    wvT = nc.dram_tensor("wvT", [H, DS], DT.bfloat16, kind="ExternalInput")
    woT = nc.dram_tensor("woT", [DS, H], DT.bfloat16, kind="ExternalInput")
    h1p = nc.dram_tensor("h1p", [S, H], DT.float32, kind="ExternalOutput")

    with tile.TileContext(nc) as tc:
        with tc.tile_pool(name="wpool", bufs=1) as wpool, \
             tc.tile_pool(name="qk", bufs=1) as qkpool, \
             tc.tile_pool(name="vp", bufs=1) as vpool, \
             tc.tile_pool(name="pt", bufs=4) as ptpool, \
             tc.tile_pool(name="ao", bufs=1) as aopool, \
             tc.tile_pool(name="rc", bufs=4) as rcpool, \
             tc.tile_pool(name="avs", bufs=3) as avspool, \
             tc.tile_pool(name="hout", bufs=4) as hpool, \
             tc.tile_pool(name="dram", bufs=2, space="DRAM") as drpool, \
             tc.tile_pool(name="pp", bufs=2, space="PSUM") as pp, \
             tc.tile_pool(name="pav", bufs=4, space="PSUM") as pav:

            # ---- load inputs, ordered by first use: early xT chunks + wq/wk
            # first, later xT chunks next, wv/wo last ----
            xts = [wpool.tile([128, S], DT.bfloat16, name=f"xt{ci}", tag=f"xt{ci}")
                   for ci in range(NCI)]
            for ci in range(3):
                nc.sync.dma_start(xts[ci][:], xT.rearrange("(c p) s -> c p s", p=128)[ci])
            wq_sb = wpool.tile([128, NCI, DS], DT.bfloat16)
            nc.sync.dma_start(wq_sb[:], wqT.rearrange("(c p) m -> p c m", p=128))
            wk_sb = wpool.tile([128, NCI, DS], DT.bfloat16)
            nc.sync.dma_start(wk_sb[:], wkT.rearrange("(c p) m -> p c m", p=128))
            for ci in range(3, NCI):
                nc.sync.dma_start(xts[ci][:], xT.rearrange("(c p) s -> c p s", p=128)[ci])
            wv_sb = wpool.tile([128, NCI, DS], DT.bfloat16)
            nc.sync.dma_start(wv_sb[:], wvT.rearrange("(c p) m -> p c m", p=128))
            wo_sb = wpool.tile([128, DS // 128, H], DT.bfloat16)
            nc.sync.dma_start(wo_sb[:], woT.rearrange("(c p) m -> p c m", p=128))

            # per-head-pair qT/kT tiles, per-head v tiles (fine dep granularity)
            qts = [qkpool.tile([64, 2, S], DT.bfloat16, name=f"q{p}", tag=f"q{p}")
                   for p in range(NHPC // 2)]
            kts = [qkpool.tile([64, 2, S], DT.bfloat16, name=f"k{p}", tag=f"k{p}")
                   for p in range(NHPC // 2)]
            vts = [vpool.tile([128, NTK, 66], DT.bfloat16, name=f"v{h}", tag=f"v{h}")
                   for h in range(NHPC)]

            def make_qk(pair, wsb, dst):
                # dst[64, 2, S] for heads (2*pair, 2*pair+1)
                for th in range(2):
                    ps = pp.tile([128, 1024], DT.float32, tag="pp", name="ps")
                    for ci in range(NCI):
                        for i, q in enumerate((2 * th, 2 * th + 1)):
                            nc.tensor.matmul(
                                ps[:, i * 512:(i + 1) * 512],
                                wsb[:, ci, pair * 128:(pair + 1) * 128],
                                xts[ci][:, q * 512:(q + 1) * 512],
                                start=(ci == 0), stop=(ci == NCI - 1),
                            )
                    nc.vector.tensor_copy(
                        dst[0:64, 0, 2 * th * 512:(2 * th + 2) * 512], ps[0:64, :])
                    nc.vector.tensor_copy(
                        dst[0:64, 1, 2 * th * 512:(2 * th + 2) * 512], ps[64:128, :])

            def make_v():
                for h in range(NHPC):
                    nc.vector.memset(vts[h][:, :, 64:66], 0.0)
                    nc.vector.memset(vts[h][:, :, 64:65], 1.0)
                for tkt in range(NTK):
                    pv = pp.tile([128, 1024], DT.float32, tag="pp", name="pv")
                    for ci in range(NCI):
                        nc.tensor.matmul(
                            pv[:, 0:DS],
                            xts[ci][:, tkt * 128:(tkt + 1) * 128],
                            wv_sb[:, ci, 0:DS],
                            start=(ci == 0), stop=(ci == NCI - 1),
                        )
                    for h in range(NHPC):
                        nc.vector.tensor_copy(
                            vts[h][:, tkt, 0:64], pv[:, h * 64:(h + 1) * 64])

            def attend(h, tqh):
                # one tq half (1024 queries) of head h
                qt, kt, vt = qts[h // 2], kts[h // 2], vts[h]
                hi = h % 2
                av = [pav.tile([65, 512], DT.float32, tag="pav", name=f"av{q}")
                      for q in range(2)]

                def av_mms(pt, tkt):
                    for i in range(2):
                        nc.tensor.matmul(
                            av[i][:],
                            vt[:, tkt, 0:65],
                            pt[:, i, :],
                            start=(tkt == 0), stop=(tkt == NTK - 1),
                        )

                # software pipeline: AV for iteration t is emitted after the
                # scores+exp of t+1, so PE never waits on ACT's exp latency.
                pending = None
                for tkt in range(NTK):
                    pt = ptpool.tile([128, 2, 512], DT.bfloat16, tag="pt")
                    sc = pp.tile([128, 1024], DT.float32, tag="pp", name="sc")
                    for i in range(2):
                        q = 2 * tqh + i
                        nc.tensor.matmul(
                            sc[:, i * 512:(i + 1) * 512],
                            kt[0:64, hi, tkt * 128:(tkt + 1) * 128],
                            qt[0:64, hi, q * 512:(q + 1) * 512],
                            start=True, stop=True,
                        )
                    nc.scalar.activation(
                        pt[:],
                        sc[:].rearrange("p (a b) -> p a b", b=512),
                        AF.Exp, scale=0.125)
                    if pending is not None:
                        av_mms(*pending)
                    pending = (pt, tkt)
                av_mms(*pending)
                # evict AV psum to SBUF right away (frees pav slots)
                av_sb = avspool.tile([65, 1024], DT.float32, tag="avs", name="av_sb")
                for i in range(2):
                    nc.vector.tensor_copy(av_sb[:, i * 512:(i + 1) * 512], av[i][:])
                rc = rcpool.tile([1, 1024], DT.float32, tag="rc")
                nc.vector.reciprocal(rc[0:1, :], av_sb[64:65, :])
                rd = drpool.tile([1, 1024], DT.float32)
                nc.sync.dma_start(rd[:], rc[:])
                rb = rcpool.tile([64, 1024], DT.float32, tag="rb")
                nc.sync.dma_start(rb[:], rd[:].to_broadcast([64, 1024]))
                roff = (h % 2) * 64
                for i in range(2):
                    q = 2 * tqh + i
                    nc.vector.tensor_tensor(
                        aoT_sb[roff:roff + 64, h // 2, q * 512:(q + 1) * 512],
                        av_sb[0:64, i * 512:(i + 1) * 512],
                        rb[:, i * 512:(i + 1) * 512],
                        ALU.mult,
                    )

            aoT_sb = aopool.tile([128, DS // 128, S], DT.bfloat16)
            make_qk(0, wq_sb, qts[0])
            make_qk(0, wk_sb, kts[0])
            make_v()
            attend(0, 0)
            make_qk(1, wq_sb, qts[1])   # overlaps ACT-bound attends of pair 0
            make_qk(1, wk_sb, kts[1])
            attend(0, 1)
            attend(1, 0)
            attend(1, 1)
            attend(2, 0)
            attend(2, 1)
            attend(3, 0)
            attend(3, 1)

            # ---- partial O-projection: h1p[t, :] = sum_o aoT[o, t] * woT[o, :] ----
            for tkt in range(NTK):
                ht = hpool.tile([128, H], DT.float32, tag="ht")
                po = pp.tile([128, 1024], DT.float32, tag="pp", name="po")
                for jc in range(H // 512):
                    for oc in range(DS // 128):
                        nc.tensor.matmul(
                            po[:, jc * 512:(jc + 1) * 512],
                            aoT_sb[:, oc, tkt * 128:(tkt + 1) * 128],
                            wo_sb[:, oc, jc * 512:(jc + 1) * 512],
                            start=(oc == 0), stop=(oc == DS // 128 - 1),
                        )
                nc.vector.tensor_copy(ht[:], po[:])
                nc.sync.dma_start(h1p[tkt * 128:(tkt + 1) * 128, :], ht[:])

    nc.compile()
    nc.finalize()
    return nc


def _build_l2(cap):
    nc = bacc.Bacc("TRN2", target_bir_lowering=False, debug=False, num_devices=NCORES)
    zeT = nc.dram_tensor("zeT", [H, cap], DT.bfloat16, kind="ExternalInput")
    w1T = nc.dram_tensor("w1T", [H, I], DT.bfloat16, kind="ExternalInput")
    w3T = nc.dram_tensor("w3T", [H, I], DT.bfloat16, kind="ExternalInput")
    w2T = nc.dram_tensor("w2T", [I, H], DT.bfloat16, kind="ExternalInput")
    web = nc.dram_tensor("web", [128, cap], DT.float32, kind="ExternalInput")
    yT = nc.dram_tensor("yT", [H, cap], DT.float32, kind="ExternalOutput")

    cch = _capacity_chunks(cap)
    NIC = I // 128
    with tile.TileContext(nc) as tc:
        with tc.tile_pool(name="wpool", bufs=1) as wpool, \
             tc.tile_pool(name="hh", bufs=1) as hhpool, \
             tc.tile_pool(name="hs", bufs=3) as hspool, \
             tc.tile_pool(name="yt", bufs=3) as ytpool, \
             tc.tile_pool(name="pg", bufs=6, space="PSUM") as pg, \
             tc.tile_pool(name="py", bufs=2, space="PSUM") as py:

            # per-chunk tiles + first-use-ordered loads so the first h-matmul
            # only waits for chunk 0 of z and w1
            zcs = [wpool.tile([128, cap], DT.bfloat16, name=f"zc{c}", tag=f"zc{c}")
                   for c in range(NCI)]
            w1cs = [wpool.tile([128, I], DT.bfloat16, name=f"w1c{c}", tag=f"w1c{c}")
                    for c in range(NCI)]
            w3cs = [wpool.tile([128, I], DT.bfloat16, name=f"w3c{c}", tag=f"w3c{c}")
                    for c in range(NCI)]
            for c in range(NCI):
                nc.sync.dma_start(zcs[c][:], zeT.rearrange("(c p) m -> c p m", p=128)[c])
                nc.sync.dma_start(w1cs[c][:], w1T.rearrange("(c p) m -> c p m", p=128)[c])
                nc.sync.dma_start(w3cs[c][:], w3T.rearrange("(c p) m -> c p m", p=128)[c])
            web_sb = wpool.tile([128, cap], DT.float32)
            nc.sync.dma_start(web_sb[:], web[:, :])

            # hh in two halves so the y-phase can start after the first half
            hhs = [hhpool.tile([128, NIC // 2, cap], DT.bfloat16, name=f"hh{a}",
                               tag=f"hh{a}") for a in range(2)]
            w2_holder = []

            for ic in range(NIC):
                hp = [pg.tile([128, 512], DT.float32, tag="pg", name=f"hp{j}")
                      for j in range(len(cch))]
                for hc in range(NCI):
                    for j, (o, ln) in enumerate(cch):
                        nc.tensor.matmul(
                            hp[j][:, 0:ln],
                            w1cs[hc][:, ic * 128:(ic + 1) * 128],
                            zcs[hc][:, o:o + ln],
                            start=(hc == 0), stop=(hc == NCI - 1),
                        )
                hs = hspool.tile([128, cap], DT.bfloat16, tag="hs", name="hs")
                for j, (o, ln) in enumerate(cch):
                    nc.scalar.activation(hs[:, o:o + ln], hp[j][:, 0:ln], AF.Silu)
                gp = [pg.tile([128, 512], DT.float32, tag="pg", name=f"gp{j}")
                      for j in range(len(cch))]
                for hc in range(NCI):
                    for j, (o, ln) in enumerate(cch):
                        nc.tensor.matmul(
                            gp[j][:, 0:ln],
                            w3cs[hc][:, ic * 128:(ic + 1) * 128],
                            zcs[hc][:, o:o + ln],
                            start=(hc == 0), stop=(hc == NCI - 1),
                        )
                for j, (o, ln) in enumerate(cch):
                    nc.vector.tensor_tensor(
                        hhs[ic // (NIC // 2)][:, ic % (NIC // 2), o:o + ln],
                        gp[j][:, 0:ln], hs[:, o:o + ln], ALU.mult)
                if ic == 0:
                    # emit w2 load after the first h-block for DMA priority
                    w2_sb = wpool.tile([128, NIC, H], DT.bfloat16)
                    nc.sync.dma_start(
                        w2_sb[:], w2T.rearrange("(c p) m -> p c m", p=128))
                    w2_holder.append(w2_sb)

            w2_sb = w2_holder[0]
            for hc in range(NCI):
                yt = ytpool.tile([128, cap], DT.float32, tag="yt", name="yt")
                for j, (o, ln) in enumerate(cch):
                    yp = py.tile([128, 512], DT.float32, tag="py", name="yp")
                    for ic in range(NIC):
                        nc.tensor.matmul(
                            yp[:, 0:ln],
                            w2_sb[:, ic, hc * 128:(hc + 1) * 128],
                            hhs[ic // (NIC // 2)][:, ic % (NIC // 2), o:o + ln],
                            start=(ic == 0), stop=(ic == NIC - 1),
                        )
                    nc.vector.tensor_tensor(
                        yt[:, o:o + ln], yp[:, 0:ln], web_sb[:, o:o + ln], ALU.mult)
                nc.sync.dma_start(yT[hc * 128:(hc + 1) * 128, :], yt[:])

    nc.compile()
    nc.finalize()
    return nc


def _get(name, builder, *args):
    if name not in _CACHE:
        _CACHE[name] = builder(*args)
    return _CACHE[name]


def _rmsnorm(x, w):
    xf = x.astype(np.float32)
    rms = 1.0 / np.sqrt((xf * xf).mean(axis=-1, keepdims=True) + EPS)
    return (xf * rms) * w.astype(np.float32)


def kernel(x, ln1_w, ln2_w, wq, wk, wv, wo, gate_w, w1, w2, w3):
    global LAST_RESULTS
    LAST_RESULTS = []
    x = np.asarray(x, np.float32)
    wq, wk, wv, wo = (np.asarray(a, np.float32) for a in (wq, wk, wv, wo))
    gate_w = np.asarray(gate_w, np.float32)
    w1, w2, w3 = (np.asarray(a, np.float32) for a in (w1, w2, w3))
    ln1_w = np.asarray(ln1_w, np.float32)
    ln2_w = np.asarray(ln2_w, np.float32)

    xf = x.reshape(T, H)
    z1 = _rmsnorm(xf, ln1_w)
    # ---- launch 1: attention ----
    nc1 = _get("l1", _build_l1)
    in_maps = []
    for c in range(NCORES):
        b, g = divmod(c, NGRP)
        sl = slice(g * DS, (g + 1) * DS)
        in_maps.append({
            "xT": np.ascontiguousarray(z1[b * S:(b + 1) * S].T).astype(BF16),
            "wqT": np.ascontiguousarray(wq[sl].T).astype(BF16),
            "wkT": np.ascontiguousarray(wk[sl].T).astype(BF16),
            "wvT": np.ascontiguousarray(wv[sl].T).astype(BF16),
            "woT": np.ascontiguousarray(wo[:, sl].T).astype(BF16),
        })
    res1 = run_bass_kernel_spmd(nc1, in_maps, core_ids=list(range(NCORES)), trace=TRACE)
    LAST_RESULTS.append(res1)

    h1 = xf.copy()
    for c in range(NCORES):
        b = c // NGRP
        h1[b * S:(b + 1) * S] += res1.results[c]["h1p"]

    # ---- host: routing (exact fp32 semantics like the reference) ----
    z = _rmsnorm(h1, ln2_w)
    logits = (z.astype(np.float64) @ gate_w.T.astype(np.float64)).astype(np.float32)
    order = np.argsort(-logits, axis=-1, kind="stable")
    sel = order[:, :2]                               # top-2, ties -> lower index
    vals = np.take_along_axis(logits, sel, axis=-1).astype(np.float32)
    mx = vals.max(axis=-1, keepdims=True)
    ex = np.exp(vals - mx)
    rw = (ex / ex.sum(axis=-1, keepdims=True)).astype(np.float32)

    idx_lists = []
    for e in range(E):
        m = (sel == e)
        tok = np.nonzero(m.any(axis=-1))[0]
        wgt = np.where(m, rw, 0.0).sum(axis=-1)[tok]
        idx_lists.append((tok, wgt.astype(np.float32)))
    maxload = max(len(tok) for tok, _ in idx_lists)
    cap = C
    while cap < maxload:
        cap += 512
    nc2 = _get(f"l2_{cap}", _build_l2, cap)

    # ---- launch 2: expert-parallel FFN ----
    zT = np.ascontiguousarray(z.T).astype(BF16)      # [H, T]
    in_maps2 = []
    for e in range(E):
        tok, wgt = idx_lists[e]
        zeT = np.zeros((H, cap), BF16)
        zeT[:, :len(tok)] = zT[:, tok]
        web = np.zeros((cap,), np.float32)
        web[:len(tok)] = wgt
        in_maps2.append({
            "zeT": zeT,
            "w1T": np.ascontiguousarray(w1[e].T).astype(BF16),
            "w3T": np.ascontiguousarray(w3[e].T).astype(BF16),
            "w2T": np.ascontiguousarray(w2[e].T).astype(BF16),
            "web": np.broadcast_to(web, (128, cap)).copy(),
        })
    res2 = run_bass_kernel_spmd(nc2, in_maps2, core_ids=list(range(NCORES)), trace=TRACE)
    LAST_RESULTS.append(res2)

    out = h1.copy()
    for e in range(E):
        tok, _ = idx_lists[e]
        out[tok] += res2.results[e]["yT"][:, :len(tok)].T

    return out.reshape(B, S, H).astype(np.float32)



# revision 7
# speedup vs baseline: 1.7389x; 1.7389x over previous
"""Trainium2 Bass kernel for a Mixtral decoder layer (attention + top-2 MoE).

Strategy (8 NeuronCores), all matmuls fp8-e4m3 with DoubleRow perf mode
(2 K-tiles contracted per instruction at 0.5 cycles/row):

  Launch 1 (attention): 2D shard = (batch b in {0,1}) x (head-group g in
    {0..3}, 4 heads / 256 feature slice each). Q/K projected into a
    [h*32+d%32, d//32, t] layout so transposed-score matmuls run as 32-row
    DoubleRow tiles (tile_position=(32h,0)). Softmax exp is split across
    three engines: ACT (exact exp, scale+bias so probs land in fp8 range)
    and DVE->Pool (Schraudolph fast-exp: int32 affine + bitcast convert).
    AV contracts 2 key-tiles per DR matmul with a ones-column folding the
    denominator; output projection on-device, host sums partials.
  Host: residual add, rmsnorm, gating logits, exact top-2 routing,
    per-expert token gather (expert-parallel dispatch in numpy).
  Launch 2 (MoE FFN): expert-parallel - core e owns expert e, processes its
    routed tokens (padded to capacity) with DR-fp8 w1/w3/w2 matmuls; silu
    via ACT (scale folds the weight upscale), gate multiply on DVE.
  Weights are upscaled (x64 / x16) so fp8-e4m3 avoids subnormals; scales
  are folded into activation scale args, psum-copy scales and host-side
  routing weights. fp32 PSUM accumulation throughout.
"""
import os
import sys

import numpy as np
import ml_dtypes

for _p in ("/root/.axon_site", "/root/.axon_site/_ro/trn_rl_repo", "/opt/trn_rl_repo"):
    if os.path.isdir(_p) and _p not in sys.path:
        sys.path.append(_p)

import concourse.tile as tile
from concourse import bacc, mybir
from concourse.bass_utils import run_bass_kernel_spmd

BF16 = ml_dtypes.bfloat16
F8 = ml_dtypes.float8_e4m3
AF = mybir.ActivationFunctionType
ALU = mybir.AluOpType
DT = mybir.dt
PM = mybir.MatmulPerfMode

H = 1024
S = 2048
B = 2
NH = 16
D = 64
E = 8
I = 2048
T = B * S
EPS = 1e-5

NCORES = 8
NGRP = 4              # head groups (cores per batch)
NHPC = NH // NGRP     # 4 heads per core
DS = NHPC * D         # 256-wide feature slice per core
NTK = S // 128        # 16 tk tiles
NQ = H // 256         # 4 DR contraction chunks (pairs of 128)

C = 1152              # MoE expert token capacity (per-expert max on this data ~1087)

WS = 64.0             # weight upscale for wq/wk/wv/wo/w1/w2
W3S = 16.0            # upscale for w3 (keeps hh = 16*silu*g under fp8 max 240)
AOS = 32.0            # attention-out staging scale (fp8 subnormal dodge)
LN16 = 2.772588722239781      # ln(16): exp bias so probs land in [~1.8, ~150]
EXPA = 1512775.3951951858     # 0.125 * 2^23 / ln2  (fast-exp scale)
EXPB = 1098421109.0           # (131 - 0.058) * 2^23 (fast-exp offset)

# exp engine split per attend (16 tk tiles): which tkts DVE+Pool handles
EXP_DVE = frozenset((2, 5, 8, 11, 13, 15))
# h1p eviction split (16 tq tiles): which go to ACT (rest DVE)
EVICT_ACT = frozenset((0, 2, 4, 6, 8, 10, 12, 14))

_CACHE = {}
LAST_RESULTS = []     # BassKernelResults of the last kernel() call (for test harness)
TRACE = os.environ.get("KERNEL_TRACE", "0") == "1"


def _capacity_chunks(cap):
    out, o = [], 0
    while o < cap:
        ln = min(512, cap - o)
        out.append((o, ln))
        o += ln
    return out


def _build_l1():
    nc = bacc.Bacc("TRN2", target_bir_lowering=False, debug=False, num_devices=NCORES)
    x8 = nc.dram_tensor("x8", [NQ, 128, 2, S], DT.float8e4, kind="ExternalInput")
    wq8 = nc.dram_tensor("wq8", [NQ, 128, 2, 2, 128], DT.float8e4, kind="ExternalInput")
    wk8 = nc.dram_tensor("wk8", [NQ, 128, 2, 2, 128], DT.float8e4, kind="ExternalInput")
    wv8 = nc.dram_tensor("wv8", [NQ, 128, 2, DS], DT.float8e4, kind="ExternalInput")
    wo8 = nc.dram_tensor("wo8", [128, 2, H], DT.float8e4, kind="ExternalInput")
    h1p = nc.dram_tensor("h1p", [S, H], DT.bfloat16, kind="ExternalOutput")

    with tile.TileContext(nc) as tc:
        with tc.tile_pool(name="wpool", bufs=1) as wpool, \
             tc.tile_pool(name="qk", bufs=1) as qkpool, \
             tc.tile_pool(name="pt", bufs=5) as ptpool, \
             tc.tile_pool(name="i32", bufs=3) as i32pool, \
             tc.tile_pool(name="rc", bufs=4) as rcpool, \
             tc.tile_pool(name="hout", bufs=3) as hpool, \
             tc.tile_pool(name="dram", bufs=2, space="DRAM") as drpool, \
             tc.tile_pool(name="pp", bufs=3, space="PSUM") as pp, \
             tc.tile_pool(name="pav", bufs=2, space="PSUM") as pav:

            # ---- input loads, ordered by first use ----
            xts = [wpool.tile([128, 2, S], DT.float8e4, name=f"xt{q}", tag=f"xt{q}")
                   for q in range(NQ)]
            for q in range(NQ):
                nc.sync.dma_start(xts[q][:], x8[q])
            wq_sb = wpool.tile([128, NQ, 2, 2, 128], DT.float8e4)
            nc.sync.dma_start(wq_sb[:], wq8.rearrange("q p j a m -> p q j a m"))
            wk_sb = wpool.tile([128, NQ, 2, 2, 128], DT.float8e4)
            nc.sync.dma_start(wk_sb[:], wk8.rearrange("q p j a m -> p q j a m"))
            wv_sb = wpool.tile([128, NQ, 2, DS], DT.float8e4)
            nc.sync.dma_start(wv_sb[:], wv8.rearrange("q p j m -> p q j m"))
            wo_sb = wpool.tile([128, 2, H], DT.float8e4)
            nc.sync.dma_start(wo_sb[:], wo8[:, :, :])
            biast = wpool.tile([128, 1], DT.float32)
            nc.vector.memset(biast[:], LN16)
            bfast = wpool.tile([128, 1], DT.float32)
            nc.vector.memset(bfast[:], EXPB)

            # wq_sb wants [p][q(kc)][jk][jq][m]; loaded as p,q,j,a,m above.
            # qt/kt layout: [h*32+d32, jq(d-half), t]
            qt = qkpool.tile([128, 2, S], DT.float8e4, name="qt", tag="qt")
            kt = qkpool.tile([128, 2, S], DT.float8e4, name="kt", tag="kt")
            # vt layout: [tk%128, r(tk pair), j(tk in pair), h*66 + (64 dims,
            # ones, pad)]
            # head stride 68 keeps the DR pair-dim stride (2*4*68=272 B)
            # a multiple of 16 (s3_lw_dual_fp8_restrictions)
            vt = qkpool.tile([128, NTK // 2, 2, 4 * 68], DT.float8e4,
                             name="vt", tag="vt")
            # aoT layout: [ds%128, ds//128, t] (ds = h*64+d feature slice)
            aoT = qkpool.tile([128, 2, S], DT.float8e4, name="aoT", tag="aoT")

            def qk_proj(wsb, dst):
                # dst[h*32+d32, jq, t] = (z1 @ w.T)[g*256 + h*64 + jq*32+d32, t]/1
                for jq in range(2):
                    for tqh in range(2):
                        ps = pp.tile([128, 1024], DT.float32, tag="pp", name="ps")
                        for half in range(2):
                            sl = slice((2 * tqh + half) * 512,
                                       (2 * tqh + half + 1) * 512)
                            for q in range(NQ):
                                nc.tensor.matmul(
                                    ps[:, half * 512:(half + 1) * 512],
                                    wsb[:, q, :, jq, :],
                                    xts[q][:, :, sl],
                                    start=(q == 0), stop=(q == NQ - 1),
                                    perf_mode=PM.DoubleRow,
                                )
                        nc.vector.tensor_scalar_mul(
                            dst[:, jq, tqh * 1024:(tqh + 1) * 1024],
                            ps[:], 1.0 / WS)

            def v_proj():
                # ones columns (and zero pad) first
                nc.vector.memset(vt[:, :, :, :].rearrange(
                    "p r j (h c) -> p (r j h) c", c=68)[:, :, 64:68], 0.0)
                nc.vector.memset(vt[:, :, :, :].rearrange(
                    "p r j (h c) -> p (r j h) c", c=68)[:, :, 64:65], 1.0)
                for grp in range(NTK // 4):   # 4 tk tiles per psum tile
                    pv = pp.tile([128, 1024], DT.float32, tag="pp", name="pv")
                    for i4 in range(4):
                        tkt = grp * 4 + i4
                        for q in range(NQ):
                            nc.tensor.matmul(
                                pv[:, i4 * 256:i4 * 256 + DS],
                                xts[q][:, :, tkt * 128:(tkt + 1) * 128],
                                wv_sb[:, q, :, :],
                                start=(q == 0), stop=(q == NQ - 1),
                                perf_mode=PM.DoubleRow,
                            )
                    # pv cols (tkt4, h, 64) -> vt[:, r0+{0,1}, j, h*66:h*66+64]
                    r0 = grp * 2
                    dst = vt[:, r0:r0 + 2, :, :].rearrange(
                        "p r j (h c) -> p (r j h) c", c=68)[:, :, 0:64]
                    src = pv[:].rearrange("p (a c) -> p a c", c=64)
                    nc.vector.tensor_scalar_mul(dst, src, 1.0 / WS)

            def attend(h, tqh):
                # one tq half (1024 queries) of head h
                hb = h * 32
                av = [pav.tile([65, 512], DT.float32, tag="pav", name=f"av{i}")
                      for i in range(2)]
                pts = []
                for r in range(NTK // 2):
                    ptt = ptpool.tile([128, 2, 1024], DT.float8e4, tag="pt")
                    pts.append(ptt)
                    for j in range(2):
                        tkt = 2 * r + j
                        sc = pp.tile([128, 1024], DT.float32, tag="pp", name="sc")
                        for i in range(2):
                            sl = slice((2 * tqh + i) * 512, (2 * tqh + i + 1) * 512)
                            nc.tensor.matmul(
                                sc[:, i * 512:(i + 1) * 512],
                                kt[hb:hb + 32, :, tkt * 128:(tkt + 1) * 128],
                                qt[hb:hb + 32, :, sl],
                                start=True, stop=True,
                                perf_mode=PM.DoubleRow,
                                tile_position=(hb, 0),
                            )
                        if tkt in EXP_DVE:
                            # Schraudolph fast-exp: DVE affine->int32, Pool
                            # bitcast-convert to fp8
                            it = i32pool.tile([128, 1024], DT.int32, tag="i32")
                            nc.vector.scalar_tensor_tensor(
                                it[:], sc[:], EXPA,
                                bfast[:].to_broadcast([128, 1024]),
                                ALU.mult, ALU.add)
                            nc.gpsimd.tensor_copy(
                                ptt[:, j, :], it[:].bitcast(DT.float32))
                        else:
                            nc.scalar.activation(
                                ptt[:, j, :], sc[:], AF.Exp,
                                scale=0.125, bias=biast[:])
                    for i in range(2):
                        nc.tensor.matmul(
                            av[i][:],
                            vt[:, r, :, h * 68:h * 68 + 65],
                            ptt[:, :, i * 512:(i + 1) * 512],
                            start=(r == 0), stop=(r == NTK // 2 - 1),
                            perf_mode=PM.DoubleRow,
                        )
                # normalize: rb = AOS / denom broadcast, aoT = av * rb
                rc = rcpool.tile([1, 1024], DT.float32, tag="rc")
                for i in range(2):
                    nc.vector.reciprocal(rc[0:1, i * 512:(i + 1) * 512],
                                         av[i][64:65, :])
                rd = drpool.tile([1, 1024], DT.float32)
                nc.sync.dma_start(rd[:], rc[:])
                rb = rcpool.tile([64, 1024], DT.float32, tag="rb")
                nc.sync.dma_start(rb[:], rd[:].to_broadcast([64, 1024]))
                ro = (h % 2) * 64
                for i in range(2):
                    sl = slice((2 * tqh + i) * 512, (2 * tqh + i + 1) * 512)
                    nc.vector.scalar_tensor_tensor(
                        aoT[ro:ro + 64, h // 2, sl],
                        av[i][0:64, :], AOS, rb[:, i * 512:(i + 1) * 512],
                        ALU.mult, ALU.mult)

            def o_proj(oc):
                # h1p rows oc*128..: psum = 2048 * true value (host rescales)
                po = pp.tile([128, 1024], DT.float32, tag="pp", name="po")
                for jc in range(2):
                    nc.tensor.matmul(
                        po[:, jc * 512:(jc + 1) * 512],
                        aoT[:, :, oc * 128:(oc + 1) * 128],
                        wo_sb[:, :, jc * 512:(jc + 1) * 512],
                        start=True, stop=True,
                        perf_mode=PM.DoubleRow,
                    )
                ht = hpool.tile([128, H], DT.bfloat16, tag="ht")
                if oc in EVICT_ACT:
                    nc.scalar.activation(ht[:], po[:], AF.Copy)
                else:
                    nc.vector.tensor_copy(ht[:], po[:])
                nc.sync.dma_start(h1p[oc * 128:(oc + 1) * 128, :], ht[:])

            qk_proj(wq_sb, qt)
            qk_proj(wk_sb, kt)
            v_proj()
            for tqh in range(2):
                for h in range(NHPC):
                    attend(h, tqh)
                for oc in range(8 * tqh, 8 * (tqh + 1)):
                    o_proj(oc)

    nc.compile()
    nc.finalize()
    return nc


def _build_l2(cap):
    nc = bacc.Bacc("TRN2", target_bir_lowering=False, debug=False, num_devices=NCORES)
    ze8 = nc.dram_tensor("ze8", [NQ, 128, 2, cap], DT.float8e4, kind="ExternalInput")
    w18 = nc.dram_tensor("w18", [NQ, 128, 2, I], DT.float8e4, kind="ExternalInput")
    w38 = nc.dram_tensor("w38", [NQ, 128, 2, I], DT.float8e4, kind="ExternalInput")
    w28 = nc.dram_tensor("w28", [I // 256, 128, 2, H], DT.float8e4,
                         kind="ExternalInput")
    web = nc.dram_tensor("web", [128, cap], DT.float32, kind="ExternalInput")
    yT = nc.dram_tensor("yT", [H, cap], DT.bfloat16, kind="ExternalOutput")

    cch = _capacity_chunks(cap)
    NIC = I // 128
    NR = I // 256
    with tile.TileContext(nc) as tc:
        with tc.tile_pool(name="wpool", bufs=1) as wpool, \
             tc.tile_pool(name="hh", bufs=1) as hhpool, \
             tc.tile_pool(name="hs", bufs=3) as hspool, \
             tc.tile_pool(name="yt", bufs=3) as ytpool, \
             tc.tile_pool(name="pg", bufs=6, space="PSUM") as pg, \
             tc.tile_pool(name="py", bufs=2, space="PSUM") as py:

            zcs = [wpool.tile([128, 2, cap], DT.float8e4, name=f"zc{q}",
                              tag=f"zc{q}") for q in range(NQ)]
            w1cs = [wpool.tile([128, 2, I], DT.float8e4, name=f"w1c{q}",
                               tag=f"w1c{q}") for q in range(NQ)]
            w3cs = [wpool.tile([128, 2, I], DT.float8e4, name=f"w3c{q}",
                               tag=f"w3c{q}") for q in range(NQ)]
            for q in range(NQ):
                nc.sync.dma_start(zcs[q][:], ze8[q])
                nc.sync.dma_start(w1cs[q][:], w18[q])
            for q in range(NQ):
                nc.sync.dma_start(w3cs[q][:], w38[q])
            web_sb = wpool.tile([128, cap], DT.float32)
            nc.sync.dma_start(web_sb[:], web[:, :])

            # hh[i%128, i//256, (i//128)%2, c] fp8, = 16*silu(z@w1)*(z@w3)
            hhs = [hhpool.tile([128, 2, cap], DT.float8e4, name=f"hh{r}",
                               tag=f"hh{r}") for r in range(NR)]
            w2_holder = []

            for ic in range(NIC):
                hp = [pg.tile([128, 512], DT.float32, tag="pg", name=f"hp{j}")
                      for j in range(len(cch))]
                for q in range(NQ):
                    for j, (o, ln) in enumerate(cch):
                        nc.tensor.matmul(
                            hp[j][:, 0:ln],
                            w1cs[q][:, :, ic * 128:(ic + 1) * 128],
                            zcs[q][:, :, o:o + ln],
                            start=(q == 0), stop=(q == NQ - 1),
                            perf_mode=PM.DoubleRow,
                        )
                hs = hspool.tile([128, cap], DT.bfloat16, tag="hs", name="hs")
                for j, (o, ln) in enumerate(cch):
                    nc.scalar.activation(hs[:, o:o + ln], hp[j][:, 0:ln],
                                         AF.Silu, scale=1.0 / WS)
                gp = [pg.tile([128, 512], DT.float32, tag="pg", name=f"gp{j}")
                      for j in range(len(cch))]
                for q in range(NQ):
                    for j, (o, ln) in enumerate(cch):
                        nc.tensor.matmul(
                            gp[j][:, 0:ln],
                            w3cs[q][:, :, ic * 128:(ic + 1) * 128],
                            zcs[q][:, :, o:o + ln],
                            start=(q == 0), stop=(q == NQ - 1),
                            perf_mode=PM.DoubleRow,
                        )
                for j, (o, ln) in enumerate(cch):
                    nc.vector.tensor_tensor(
                        hhs[ic // 2][:, ic % 2, o:o + ln],
                        gp[j][:, 0:ln], hs[:, o:o + ln], ALU.mult)
                if ic == 0:
                    # emit w2 load after the first h-block for DMA priority
                    w2_sb = wpool.tile([128, NR, 2, H], DT.float8e4)
                    nc.sync.dma_start(w2_sb[:], w28.rearrange("r p j m -> p r j m"))
                    w2_holder.append(w2_sb)

            w2_sb = w2_holder[0]
            for hc in range(H // 128):
                yt = ytpool.tile([128, cap], DT.bfloat16, tag="yt", name="yt")
                for j, (o, ln) in enumerate(cch):
                    yp = py.tile([128, 512], DT.float32, tag="py", name="yp")
                    for r in range(NR):
                        nc.tensor.matmul(
                            yp[:, 0:ln],
                            w2_sb[:, r, :, hc * 128:(hc + 1) * 128],
                            hhs[r][:, :, o:o + ln],
                            start=(r == 0), stop=(r == NR - 1),
                            perf_mode=PM.DoubleRow,
                        )
                    nc.vector.tensor_tensor(
                        yt[:, o:o + ln], yp[:, 0:ln], web_sb[:, o:o + ln],
                        ALU.mult)
                nc.sync.dma_start(yT[hc * 128:(hc + 1) * 128, :], yt[:])

    nc.compile()
    nc.finalize()
    return nc


def _get(name, builder, *args):
    if name not in _CACHE:
        _CACHE[name] = builder(*args)
    return _CACHE[name]


def _rmsnorm(x, w):
    xf = x.astype(np.float32)
    rms = 1.0 / np.sqrt((xf * xf).mean(axis=-1, keepdims=True) + EPS)
    return (xf * rms) * w.astype(np.float32)


def _f8(a):
    return np.clip(a, -240.0, 240.0).astype(F8)


def _qpack(mat_T):
    """[H, M] -> [H//256, 128, 2, M] with row (q*2+j)*128+p -> [q, p, j, :]."""
    M = mat_T.shape[1]
    return np.ascontiguousarray(
        mat_T.reshape(NQ, 2, 128, M).transpose(0, 2, 1, 3))


def kernel(x, ln1_w, ln2_w, wq, wk, wv, wo, gate_w, w1, w2, w3):
    global LAST_RESULTS
    LAST_RESULTS = []
    x = np.asarray(x, np.float32)
    wq, wk, wv, wo = (np.asarray(a, np.float32) for a in (wq, wk, wv, wo))
    gate_w = np.asarray(gate_w, np.float32)
    w1, w2, w3 = (np.asarray(a, np.float32) for a in (w1, w2, w3))
    ln1_w = np.asarray(ln1_w, np.float32)
    ln2_w = np.asarray(ln2_w, np.float32)

    xf = x.reshape(T, H)
    z1 = _rmsnorm(xf, ln1_w)
    # ---- launch 1: attention ----
    nc1 = _get("l1", _build_l1)
    in_maps = []
    for c in range(NCORES):
        b, g = divmod(c, NGRP)
        x8 = _f8(_qpack(np.ascontiguousarray(z1[b * S:(b + 1) * S].T)))
        # wq8[q, p, jk, jq, h*32+d] = wq[g*256 + h*64 + jq*32 + d, (q*2+jk)*128+p]
        wqg = wq[g * DS:(g + 1) * DS] * WS   # [256, H]
        wkg = wk[g * DS:(g + 1) * DS] * WS
        wvg = wv[g * DS:(g + 1) * DS] * WS
        wog = wo[:, g * DS:(g + 1) * DS] * WS  # [H, 256]

        def _qk_pack(w):
            a = _qpack(np.ascontiguousarray(w.T))       # [q, p, jk, 256]
            a = a.reshape(NQ, 128, 2, NHPC, 2, 32)       # f = h*64+jq*32+d
            return _f8(np.ascontiguousarray(
                a.transpose(0, 1, 2, 4, 3, 5).reshape(NQ, 128, 2, 2, 128)))

        wo8 = np.ascontiguousarray(
            wog.T.reshape(2, 128, H).transpose(1, 0, 2))  # [p, j, H]
        in_maps.append({
            "x8": x8,
            "wq8": _qk_pack(wqg),
            "wk8": _qk_pack(wkg),
            "wv8": _f8(_qpack(np.ascontiguousarray(wvg.T))),
            "wo8": _f8(wo8),
        })
    res1 = run_bass_kernel_spmd(nc1, in_maps, core_ids=list(range(NCORES)),
                                trace=TRACE)
    LAST_RESULTS.append(res1)

    h1 = xf.copy()
    for c in range(NCORES):
        b = c // NGRP
        h1[b * S:(b + 1) * S] += res1.results[c]["h1p"].astype(np.float32) \
            / (AOS * WS)

    # ---- host: routing (exact fp32 semantics like the reference) ----
    z = _rmsnorm(h1, ln2_w)
    logits = (z.astype(np.float64) @ gate_w.T.astype(np.float64)).astype(np.float32)
    order = np.argsort(-logits, axis=-1, kind="stable")
    sel = order[:, :2]                               # top-2, ties -> lower index
    vals = np.take_along_axis(logits, sel, axis=-1).astype(np.float32)
    mx = vals.max(axis=-1, keepdims=True)
    ex = np.exp(vals - mx)
    rw = (ex / ex.sum(axis=-1, keepdims=True)).astype(np.float32)

    idx_lists = []
    for e in range(E):
        m = (sel == e)
        tok = np.nonzero(m.any(axis=-1))[0]
        wgt = np.where(m, rw, 0.0).sum(axis=-1)[tok]
        idx_lists.append((tok, wgt.astype(np.float32)))
    maxload = max(len(tok) for tok, _ in idx_lists)
    cap = C
    while cap < maxload:
        cap += 512
    nc2 = _get(f"l2_{cap}", _build_l2, cap)

    # ---- launch 2: expert-parallel FFN ----
    zT = np.ascontiguousarray(z.T)                   # [H, T] fp32
    in_maps2 = []
    for e in range(E):
        tok, wgt = idx_lists[e]
        zeT = np.zeros((H, cap), np.float32)
        zeT[:, :len(tok)] = zT[:, tok]
        web = np.zeros((cap,), np.float32)
        web[:len(tok)] = wgt / (WS * W3S)
        in_maps2.append({
            "ze8": _f8(_qpack(zeT)),
            "w18": _f8(_qpack(np.ascontiguousarray(w1[e].T)) * WS),
            "w38": _f8(_qpack(np.ascontiguousarray(w3[e].T)) * W3S),
            "w28": _f8(np.ascontiguousarray(
                w2[e].T.reshape(I // 256, 2, 128, H).transpose(0, 2, 1, 3)) * WS),
            "web": np.broadcast_to(web, (128, cap)).copy(),
        })
    res2 = run_bass_kernel_spmd(nc2, in_maps2, core_ids=list(range(NCORES)),
                                trace=TRACE)
    LAST_RESULTS.append(res2)

    out = h1.copy()
    for e in range(E):
        tok, _ = idx_lists[e]
        out[tok] += res2.results[e]["yT"][:, :len(tok)].T.astype(np.float32)

    return out.reshape(B, S, H).astype(np.float32)


# revision 18
# speedup vs baseline: 1.9450x; 1.1185x over previous
"""Trainium2 Bass kernel for a Mixtral decoder layer (attention + top-2 MoE).

Strategy (8 NeuronCores), all matmuls fp8-e4m3 with DoubleRow perf mode
(2 K-tiles contracted per instruction at 0.5 cycles/row):

  Launch 1 (attention): 2D shard = (batch b in {0,1}) x (head-group g in
    {0..3}, 4 heads / 256 feature slice each). Q/K projected into a
    [h*32+d%32, d//32, t] layout so transposed-score matmuls run as 32-row
    DoubleRow tiles (tile_position=(32h,0)). Softmax exp is split across
    three engines: ACT (exact exp, scale+bias so probs land in fp8 range)
    and DVE->Pool (Schraudolph fast-exp: int32 affine + bitcast convert).
    AV contracts 2 key-tiles per DR matmul with a ones-column folding the
    denominator; output projection on-device, host sums partials.
  Host: residual add, rmsnorm, gating logits, exact top-2 routing,
    per-expert token gather (expert-parallel dispatch in numpy).
  Launch 2 (MoE FFN): expert-parallel - core e owns expert e, processes its
    routed tokens (padded to capacity) with DR-fp8 w1/w3/w2 matmuls; silu
    via ACT (scale folds the weight upscale), gate multiply on DVE.
  Weights are upscaled (x64 / x16) so fp8-e4m3 avoids subnormals; scales
  are folded into activation scale args, psum-copy scales and host-side
  routing weights. fp32 PSUM accumulation throughout.
"""
import os
import sys

import numpy as np
import ml_dtypes

for _p in ("/root/.axon_site", "/root/.axon_site/_ro/trn_rl_repo", "/opt/trn_rl_repo"):
    if os.path.isdir(_p) and _p not in sys.path:
        sys.path.append(_p)

import concourse.tile as tile
from concourse import bacc, mybir
from concourse.bass_utils import run_bass_kernel_spmd

BF16 = ml_dtypes.bfloat16
F8 = ml_dtypes.float8_e4m3
AF = mybir.ActivationFunctionType
ALU = mybir.AluOpType
DT = mybir.dt
PM = mybir.MatmulPerfMode

H = 1024
S = 2048
B = 2
NH = 16
D = 64
E = 8
I = 2048
T = B * S
EPS = 1e-5

NCORES = 8
NGRP = 4              # head groups (cores per batch)
NHPC = NH // NGRP     # 4 heads per core
DS = NHPC * D         # 256-wide feature slice per core
NTK = S // 128        # 16 tk tiles
NQ = H // 256         # 4 DR contraction chunks (pairs of 128)

C = 1152              # MoE expert token capacity (per-expert max on this data ~1087)

WS = 64.0             # weight upscale for wq/wk/wv/wo/w1/w2
W3S = 16.0            # upscale for w3 (keeps hh = 16*silu*g under fp8 max 240)
AOS = 32.0            # attention-out staging scale (fp8 subnormal dodge)
LN16 = 2.772588722239781      # ln(16): exp bias so probs land in [~1.8, ~150]
EXPA = 1512775.3951951858     # 0.125 * 2^23 / ln2  (fast-exp scale)
EXPB = 1098421109.0           # (131 - 0.058) * 2^23 (fast-exp offset)

# exp column split per score tile [128,1024]: ACT takes [0:EXPQ), DVE+Pool rest
EXPQ = 736
# h1p eviction split (16 tq tiles): which go to ACT (rest DVE)
EVICT_ACT = frozenset((0, 2, 4, 6, 8, 10, 12, 14))

_CACHE = {}
LAST_RESULTS = []     # BassKernelResults of the last kernel() call (for test harness)
TRACE = os.environ.get("KERNEL_TRACE", "0") == "1"


def _capacity_chunks(cap):
    out, o = [], 0
    while o < cap:
        ln = min(512, cap - o)
        out.append((o, ln))
        o += ln
    return out


def _build_l1():
    nc = bacc.Bacc("TRN2", target_bir_lowering=False, debug=False, num_devices=NCORES)
    x8 = nc.dram_tensor("x8", [NQ, 128, 2, S], DT.float8e4, kind="ExternalInput")
    wq8 = nc.dram_tensor("wq8", [NQ, 128, 2, 2, 128], DT.float8e4, kind="ExternalInput")
    wk8 = nc.dram_tensor("wk8", [NQ, 128, 2, 2, 128], DT.float8e4, kind="ExternalInput")
    wv8 = nc.dram_tensor("wv8", [NQ, 128, 2, DS], DT.float8e4, kind="ExternalInput")
    wo8 = nc.dram_tensor("wo8", [128, 2, H], DT.float8e4, kind="ExternalInput")
    h1p = nc.dram_tensor("h1p", [S, H], DT.bfloat16, kind="ExternalOutput")

    with tile.TileContext(nc) as tc:
        with tc.tile_pool(name="wpool", bufs=1) as wpool, \
             tc.tile_pool(name="qk", bufs=1) as qkpool, \
             tc.tile_pool(name="pt", bufs=6) as ptpool, \
             tc.tile_pool(name="i32", bufs=3) as i32pool, \
             tc.tile_pool(name="rc", bufs=4) as rcpool, \
             tc.tile_pool(name="hout", bufs=3) as hpool, \
             tc.tile_pool(name="dram", bufs=2, space="DRAM") as drpool, \
             tc.tile_pool(name="pp", bufs=3, space="PSUM") as pp, \
             tc.tile_pool(name="pav", bufs=2, space="PSUM") as pav:

            # ---- input loads, ordered by first use ----
            xts = [wpool.tile([128, 2, S], DT.float8e4, name=f"xt{q}", tag=f"xt{q}")
                   for q in range(NQ)]
            nc.sync.dma_start(xts[0][:], x8[0])
            wq_sb = wpool.tile([128, NQ, 2, 2, 128], DT.float8e4)
            nc.sync.dma_start(wq_sb[:], wq8.rearrange("q p j a m -> p q j a m"))
            wk_sb = wpool.tile([128, NQ, 2, 2, 128], DT.float8e4)
            nc.sync.dma_start(wk_sb[:], wk8.rearrange("q p j a m -> p q j a m"))
            for q in range(1, NQ):
                nc.sync.dma_start(xts[q][:], x8[q])
            wv_sb = wpool.tile([128, NQ, 2, DS], DT.float8e4)
            nc.sync.dma_start(wv_sb[:], wv8.rearrange("q p j m -> p q j m"))
            wo_sb = wpool.tile([128, 2, H], DT.float8e4)
            nc.sync.dma_start(wo_sb[:], wo8[:, :, :])
            biast = wpool.tile([128, 1], DT.float32)
            nc.vector.memset(biast[:], LN16)
            bfast = wpool.tile([128, 1], DT.float32)
            nc.vector.memset(bfast[:], EXPB)

            # PE warmup: dummy matmuls on memset tiles during the DMA wait
            # so the projection matmuls run at full clock (HAM ramp).
            wa = wpool.tile([128, 64], DT.bfloat16)
            nc.vector.memset(wa[:], 0.0)
            wb = wpool.tile([128, 512], DT.bfloat16)
            nc.vector.memset(wb[:], 0.0)
            wp = pp.tile([128, 512], DT.float32, tag="pp", name="warm")
            for _ in range(8):
                nc.tensor.matmul(wp[0:64, :], wa[:], wb[:],
                                 start=True, stop=True)
            wsink = wpool.tile([64, 64], DT.bfloat16)
            nc.vector.tensor_copy(wsink[:], wp[0:64, 0:64])

            # qt/kt layout: [h*32+d32, jq(d-half), t]
            qt = qkpool.tile([128, 2, S], DT.float8e4, name="qt", tag="qt")
            kt = qkpool.tile([128, 2, S], DT.float8e4, name="kt", tag="kt")
            # vt layout: [tk%128, r(tk pair), j(tk in pair), h*68 + (64 dims,
            # ones, pad)] - head stride 68 keeps the DR pair-dim stride
            # (2*4*68=272 B) a multiple of 16 (s3_lw_dual_fp8_restrictions)
            vt = qkpool.tile([128, NTK // 2, 2, 4 * 68], DT.float8e4,
                             name="vt", tag="vt")
            # aoT layout: [ds%128, ds//128, t] (ds = h*64+d feature slice)
            aoT = qkpool.tile([128, 2, S], DT.float8e4, name="aoT", tag="aoT")

            def qk_proj_unit(wsb, dst, jq, tqh, eng="dve"):
                # dst[h*32+d32, jq, t] = (z1 @ w.T)[g*256 + h*64 + jq*32+d32, t]
                ps = pp.tile([128, 1024], DT.float32, tag="pp", name="ps")
                for half in range(2):
                    sl = slice((2 * tqh + half) * 512,
                               (2 * tqh + half + 1) * 512)
                    for q in range(NQ):
                        nc.tensor.matmul(
                            ps[:, half * 512:(half + 1) * 512],
                            wsb[:, q, :, jq, :],
                            xts[q][:, :, sl],
                            start=(q == 0), stop=(q == NQ - 1),
                            perf_mode=PM.DoubleRow,
                        )
                d = dst[:, jq, tqh * 1024:(tqh + 1) * 1024]
                if eng == "act":
                    nc.scalar.activation(d, ps[:], AF.Copy, scale=1.0 / WS)
                else:
                    nc.vector.tensor_scalar_mul(d, ps[:], 1.0 / WS)

            def v_init():
                # ones columns (and zero pad)
                nc.vector.memset(vt[:, :, :, :].rearrange(
                    "p r j (h c) -> p (r j h) c", c=68)[:, :, 64:68], 0.0)
                nc.vector.memset(vt[:, :, :, :].rearrange(
                    "p r j (h c) -> p (r j h) c", c=68)[:, :, 64:65], 1.0)

            def v_group(grp):
                # 4 tk tiles per psum tile
                pv = pp.tile([128, 1024], DT.float32, tag="pp", name="pv")
                for i4 in range(4):
                    tkt = grp * 4 + i4
                    for q in range(NQ):
                        nc.tensor.matmul(
                            pv[:, i4 * 256:i4 * 256 + DS],
                            xts[q][:, :, tkt * 128:(tkt + 1) * 128],
                            wv_sb[:, q, :, :],
                            start=(q == 0), stop=(q == NQ - 1),
                            perf_mode=PM.DoubleRow,
                        )
                # pv cols (tkt4, h, 64) -> vt[:, r0+{0,1}, j, h*68:+64]
                r0 = grp * 2
                dst = vt[:, r0:r0 + 2, :, :].rearrange(
                    "p r j (h c) -> p (r j h) c", c=68)[:, :, 0:64]
                nc.vector.tensor_scalar_mul(
                    dst, pv[:].rearrange("p (a c) -> p a c", c=64), 1.0 / WS)

            def core_open(h):
                return [pav.tile([65, 512], DT.float32, tag="pav",
                                 name=f"av{h}_{i}") for i in range(2)]

            def av_mms(h, av, ptt, r):
                for i in range(2):
                    nc.tensor.matmul(
                        av[i][:],
                        vt[:, r, :, h * 68:h * 68 + 65],
                        ptt[:, :, i * 512:(i + 1) * 512],
                        start=(r == 0), stop=(r == NTK // 2 - 1),
                        perf_mode=PM.DoubleRow,
                    )

            def core_step(h, tqh, r, av, pend):
                # scores + exp for tk pair r of head h (one tq half); the AV
                # matmuls for the PREVIOUS pair are emitted after this pair's
                # scores so PE never head-of-line blocks on exp latency.
                # exp is column-split: ACT handles [0:EXPQ) (exact exp),
                # DVE->Pool handles [EXPQ:1024) (Schraudolph fast-exp).
                hb = h * 32
                ptt = ptpool.tile([128, 2, 1024], DT.float8e4, tag="pt")
                for j in range(2):
                    tkt = 2 * r + j
                    sc = pp.tile([128, 1024], DT.float32, tag="pp", name="sc")
                    for i in range(2):
                        sl = slice((2 * tqh + i) * 512, (2 * tqh + i + 1) * 512)
                        nc.tensor.matmul(
                            sc[:, i * 512:(i + 1) * 512],
                            kt[hb:hb + 32, :, tkt * 128:(tkt + 1) * 128],
                            qt[hb:hb + 32, :, sl],
                            start=True, stop=True,
                            perf_mode=PM.DoubleRow,
                            tile_position=(hb, 0),
                        )
                    nc.scalar.activation(
                        ptt[:, j, 0:EXPQ], sc[:, 0:EXPQ], AF.Exp,
                        scale=0.125, bias=biast[:])
                    it = i32pool.tile([128, 1024 - EXPQ], DT.int32, tag="i32")
                    nc.vector.scalar_tensor_tensor(
                        it[:], sc[:, EXPQ:1024], EXPA,
                        bfast[:].to_broadcast([128, 1024 - EXPQ]),
                        ALU.mult, ALU.add)
                    nc.gpsimd.tensor_copy(
                        ptt[:, j, EXPQ:1024], it[:].bitcast(DT.float32))
                if pend is not None:
                    av_mms(h, av, *pend)
                return (ptt, r)

            def norm_recip(av):
                # rb = 1/denom broadcast to 64 partitions via a DMA round-trip
                # (off-engine; latency is hidden by the deferred apply)
                rc = rcpool.tile([1, 1024], DT.bfloat16, tag="rc")
                with nc.allow_low_precision(reason="softmax denom recip, bf16"):
                    for i in range(2):
                        nc.vector.reciprocal(rc[0:1, i * 512:(i + 1) * 512],
                                             av[i][64:65, :])
                rd = drpool.tile([1, 1024], DT.bfloat16)
                nc.sync.dma_start(rd[:], rc[:])
                rb = rcpool.tile([64, 1024], DT.bfloat16, tag="rb")
                nc.sync.dma_start(rb[:], rd[:].to_broadcast([64, 1024]))
                return rb

            def norm_apply(h, tqh, av, rb):
                ro = (h % 2) * 64
                for i in range(2):
                    sl = slice((2 * tqh + i) * 512, (2 * tqh + i + 1) * 512)
                    nc.vector.scalar_tensor_tensor(
                        aoT[ro:ro + 64, h // 2, sl],
                        av[i][0:64, :], AOS, rb[:, i * 512:(i + 1) * 512],
                        ALU.mult, ALU.mult)

            def o_proj(oc):
                # h1p rows oc*128..: psum = 2048 * true value (host rescales)
                po = pp.tile([128, 1024], DT.float32, tag="pp", name="po")
                for jc in range(2):
                    nc.tensor.matmul(
                        po[:, jc * 512:(jc + 1) * 512],
                        aoT[:, :, oc * 128:(oc + 1) * 128],
                        wo_sb[:, :, jc * 512:(jc + 1) * 512],
                        start=True, stop=True,
                        perf_mode=PM.DoubleRow,
                    )
                ht = hpool.tile([128, H], DT.bfloat16, tag="ht")
                nc.scalar.activation(ht[:, 0:512], po[:, 0:512], AF.Copy)
                nc.vector.tensor_copy(ht[:, 512:1024], po[:, 512:1024])
                nc.sync.dma_start(h1p[oc * 128:(oc + 1) * 128, :], ht[:])

            def attend(h, tqh, prev, inject=None):
                # prev = (h', tqh', av', rc') of the previous attend; its
                # broadcast+apply is emitted after step 0 so the PE FIFO never
                # blocks on the reciprocal, and the av-buffer WAR for this
                # attend's lagged AV (emitted from step 1) stays safe.
                av = core_open(h)
                pend = None
                for r in range(NTK // 2):
                    pend = core_step(h, tqh, r, av, pend)
                    if r == 0 and prev is not None:
                        norm_apply(*prev)
                    if inject and r in inject:
                        inject[r]()
                av_mms(h, av, *pend)
                return (h, tqh, av, norm_recip(av))

            # ---- schedule ----
            v_init()
            qk_proj_unit(wq_sb, qt, 0, 0, "dve")
            qk_proj_unit(wq_sb, qt, 1, 0, "act")
            qk_proj_unit(wk_sb, kt, 0, 0, "dve")
            qk_proj_unit(wk_sb, kt, 1, 0, "act")

            prev = attend(0, 0, None, inject={
                0: lambda: v_group(0),
                1: lambda: v_group(1),
                2: lambda: v_group(2),
                3: lambda: v_group(3),
                4: lambda: qk_proj_unit(wq_sb, qt, 0, 1, "act"),
                6: lambda: qk_proj_unit(wq_sb, qt, 1, 1, "dve"),
            })
            prev = attend(1, 0, prev, inject={
                1: lambda: qk_proj_unit(wk_sb, kt, 0, 1, "act"),
                5: lambda: qk_proj_unit(wk_sb, kt, 1, 1, "dve"),
            })
            prev = attend(2, 0, prev)
            prev = attend(3, 0, prev)
            prev = attend(0, 1, prev)
            prev = attend(1, 1, prev,
                          inject={2: lambda: o_proj(0), 5: lambda: o_proj(1)})
            prev = attend(2, 1, prev,
                          inject={2: lambda: o_proj(2), 4: lambda: o_proj(3),
                                  6: lambda: o_proj(4)})
            prev = attend(3, 1, prev,
                          inject={2: lambda: o_proj(5), 4: lambda: o_proj(6),
                                  6: lambda: o_proj(7)})
            norm_apply(*prev)
            for oc in range(8, 16):
                o_proj(oc)

    nc.compile()
    nc.finalize()
    return nc


def _build_l2(cap):
    nc = bacc.Bacc("TRN2", target_bir_lowering=False, debug=False, num_devices=NCORES)
    ze8 = nc.dram_tensor("ze8", [NQ, 128, 2, cap], DT.float8e4, kind="ExternalInput")
    w18 = nc.dram_tensor("w18", [NQ, 128, 2, I], DT.float8e4, kind="ExternalInput")
    w38 = nc.dram_tensor("w38", [NQ, 128, 2, I], DT.float8e4, kind="ExternalInput")
    w28 = nc.dram_tensor("w28", [I // 256, 128, 2, H], DT.float8e4,
                         kind="ExternalInput")
    web = nc.dram_tensor("web", [128, cap], DT.float32, kind="ExternalInput")
    yT = nc.dram_tensor("yT", [H, cap], DT.bfloat16, kind="ExternalOutput")

    cch = _capacity_chunks(cap)
    NIC = I // 128
    NR = I // 256
    with tile.TileContext(nc) as tc:
        with tc.tile_pool(name="wpool", bufs=1) as wpool, \
             tc.tile_pool(name="hh", bufs=1) as hhpool, \
             tc.tile_pool(name="hs", bufs=3) as hspool, \
             tc.tile_pool(name="yt", bufs=3) as ytpool, \
             tc.tile_pool(name="pg", bufs=8, space="PSUM") as pg:
            py = pg

            zcs = [wpool.tile([128, 2, cap], DT.float8e4, name=f"zc{q}",
                              tag=f"zc{q}") for q in range(NQ)]
            w1cs = [wpool.tile([128, 2, I], DT.float8e4, name=f"w1c{q}",
                               tag=f"w1c{q}") for q in range(NQ)]
            w3cs = [wpool.tile([128, 2, I], DT.float8e4, name=f"w3c{q}",
                               tag=f"w3c{q}") for q in range(NQ)]
            zh = cap // 2
            for q in range(NQ):
                nc.sync.dma_start(zcs[q][:, :, 0:zh], ze8[q][:, :, 0:zh])
                nc.sync.dma_start(w1cs[q][:, :, 0:256], w18[q][:, :, 0:256])
            for q in range(NQ):
                nc.sync.dma_start(zcs[q][:, :, zh:cap], ze8[q][:, :, zh:cap])
                nc.sync.dma_start(w1cs[q][:, :, 256:I], w18[q][:, :, 256:I])
            for q in range(NQ):
                nc.sync.dma_start(w3cs[q][:, :, 0:256], w38[q][:, :, 0:256])
                nc.sync.dma_start(w3cs[q][:, :, 256:I], w38[q][:, :, 256:I])
            web_sb = wpool.tile([128, cap], DT.float32)
            nc.sync.dma_start(web_sb[:], web[:, :])

            # hh[i%128, i//256, (i//128)%2, c] fp8, = 16*silu(z@w1)*(z@w3)
            hhs = [hhpool.tile([128, 2, cap], DT.float8e4, name=f"hh{r}",
                               tag=f"hh{r}") for r in range(NR)]
            w2_holder = []

            for ic in range(NIC):
                hp = [pg.tile([128, 512], DT.float32, tag="pg", name=f"hp{j}")
                      for j in range(len(cch))]
                for q in range(NQ):
                    for j, (o, ln) in enumerate(cch):
                        nc.tensor.matmul(
                            hp[j][:, 0:ln],
                            w1cs[q][:, :, ic * 128:(ic + 1) * 128],
                            zcs[q][:, :, o:o + ln],
                            start=(q == 0), stop=(q == NQ - 1),
                            perf_mode=PM.DoubleRow,
                        )
                hs = hspool.tile([128, cap], DT.bfloat16, tag="hs", name="hs")
                for j, (o, ln) in enumerate(cch):
                    nc.scalar.activation(hs[:, o:o + ln], hp[j][:, 0:ln],
                                         AF.Silu, scale=1.0 / WS)
                gp = [pg.tile([128, 512], DT.float32, tag="pg", name=f"gp{j}")
                      for j in range(len(cch))]
                for q in range(NQ):
                    for j, (o, ln) in enumerate(cch):
                        nc.tensor.matmul(
                            gp[j][:, 0:ln],
                            w3cs[q][:, :, ic * 128:(ic + 1) * 128],
                            zcs[q][:, :, o:o + ln],
                            start=(q == 0), stop=(q == NQ - 1),
                            perf_mode=PM.DoubleRow,
                        )
                for j, (o, ln) in enumerate(cch):
                    nc.vector.tensor_tensor(
                        hhs[ic // 2][:, ic % 2, o:o + ln],
                        gp[j][:, 0:ln], hs[:, o:o + ln], ALU.mult)
                if ic == 0:
                    # emit w2 load after the first h-block for DMA priority
                    w2_sb = wpool.tile([128, NR, 2, H], DT.float8e4)
                    nc.sync.dma_start(w2_sb[:], w28.rearrange("r p j m -> p r j m"))
                    w2_holder.append(w2_sb)

            w2_sb = w2_holder[0]
            for hc in range(H // 128):
                yt = ytpool.tile([128, cap], DT.bfloat16, tag="yt", name="yt")
                for j, (o, ln) in enumerate(cch):
                    yp = py.tile([128, 512], DT.float32, tag="pg", name="yp")
                    for r in range(NR):
                        nc.tensor.matmul(
                            yp[:, 0:ln],
                            w2_sb[:, r, :, hc * 128:(hc + 1) * 128],
                            hhs[r][:, :, o:o + ln],
                            start=(r == 0), stop=(r == NR - 1),
                            perf_mode=PM.DoubleRow,
                        )
                    nc.vector.tensor_tensor(
                        yt[:, o:o + ln], yp[:, 0:ln], web_sb[:, o:o + ln],
                        ALU.mult)
                nc.sync.dma_start(yT[hc * 128:(hc + 1) * 128, :], yt[:])

    nc.compile()
    nc.finalize()
    return nc


def _get(name, builder, *args):
    if name not in _CACHE:
        _CACHE[name] = builder(*args)
    return _CACHE[name]


def _rmsnorm(x, w):
    xf = x.astype(np.float32)
    rms = 1.0 / np.sqrt((xf * xf).mean(axis=-1, keepdims=True) + EPS)
    return (xf * rms) * w.astype(np.float32)


def _f8(a):
    return np.clip(a, -240.0, 240.0).astype(F8)


def _qpack(mat_T):
    """[H, M] -> [H//256, 128, 2, M] with row (q*2+j)*128+p -> [q, p, j, :]."""
    M = mat_T.shape[1]
    return np.ascontiguousarray(
        mat_T.reshape(NQ, 2, 128, M).transpose(0, 2, 1, 3))


def kernel(x, ln1_w, ln2_w, wq, wk, wv, wo, gate_w, w1, w2, w3):
    global LAST_RESULTS
    LAST_RESULTS = []
    x = np.asarray(x, np.float32)
    wq, wk, wv, wo = (np.asarray(a, np.float32) for a in (wq, wk, wv, wo))
    gate_w = np.asarray(gate_w, np.float32)
    w1, w2, w3 = (np.asarray(a, np.float32) for a in (w1, w2, w3))
    ln1_w = np.asarray(ln1_w, np.float32)
    ln2_w = np.asarray(ln2_w, np.float32)

    xf = x.reshape(T, H)
    z1 = _rmsnorm(xf, ln1_w)
    # ---- launch 1: attention ----
    nc1 = _get("l1", _build_l1)
    in_maps = []
    for c in range(NCORES):
        b, g = divmod(c, NGRP)
        x8 = _f8(_qpack(np.ascontiguousarray(z1[b * S:(b + 1) * S].T)))
        # wq8[q, p, jk, jq, h*32+d] = wq[g*256 + h*64 + jq*32 + d, (q*2+jk)*128+p]
        wqg = wq[g * DS:(g + 1) * DS] * WS   # [256, H]
        wkg = wk[g * DS:(g + 1) * DS] * WS
        wvg = wv[g * DS:(g + 1) * DS] * WS
        wog = wo[:, g * DS:(g + 1) * DS] * WS  # [H, 256]

        def _qk_pack(w):
            a = _qpack(np.ascontiguousarray(w.T))       # [q, p, jk, 256]
            a = a.reshape(NQ, 128, 2, NHPC, 2, 32)       # f = h*64+jq*32+d
            return _f8(np.ascontiguousarray(
                a.transpose(0, 1, 2, 4, 3, 5).reshape(NQ, 128, 2, 2, 128)))

        wo8 = np.ascontiguousarray(
            wog.T.reshape(2, 128, H).transpose(1, 0, 2))  # [p, j, H]
        in_maps.append({
            "x8": x8,
            "wq8": _qk_pack(wqg),
            "wk8": _qk_pack(wkg),
            "wv8": _f8(_qpack(np.ascontiguousarray(wvg.T))),
            "wo8": _f8(wo8),
        })
    res1 = run_bass_kernel_spmd(nc1, in_maps, core_ids=list(range(NCORES)),
                                trace=TRACE)
    LAST_RESULTS.append(res1)

    h1 = xf.copy()
    for c in range(NCORES):
        b = c // NGRP
        h1[b * S:(b + 1) * S] += res1.results[c]["h1p"].astype(np.float32) \
            / (AOS * WS)

    # ---- host: routing (exact fp32 semantics like the reference) ----
    z = _rmsnorm(h1, ln2_w)
    logits = (z.astype(np.float64) @ gate_w.T.astype(np.float64)).astype(np.float32)
    order = np.argsort(-logits, axis=-1, kind="stable")
    sel = order[:, :2]                               # top-2, ties -> lower index
    vals = np.take_along_axis(logits, sel, axis=-1).astype(np.float32)
    mx = vals.max(axis=-1, keepdims=True)
    ex = np.exp(vals - mx)
    rw = (ex / ex.sum(axis=-1, keepdims=True)).astype(np.float32)

    idx_lists = []
    for e in range(E):
        m = (sel == e)
        tok = np.nonzero(m.any(axis=-1))[0]
        wgt = np.where(m, rw, 0.0).sum(axis=-1)[tok]
        idx_lists.append((tok, wgt.astype(np.float32)))
    maxload = max(len(tok) for tok, _ in idx_lists)
    cap = C
    while cap < maxload:
        cap += 512
    nc2 = _get(f"l2_{cap}", _build_l2, cap)

    # ---- launch 2: expert-parallel FFN ----
    zT = np.ascontiguousarray(z.T)                   # [H, T] fp32
    in_maps2 = []
    for e in range(E):
        tok, wgt = idx_lists[e]
        zeT = np.zeros((H, cap), np.float32)
        zeT[:, :len(tok)] = zT[:, tok]
        web = np.zeros((cap,), np.float32)
        web[:len(tok)] = wgt / (WS * W3S)
        in_maps2.append({
            "ze8": _f8(_qpack(zeT)),
            "w18": _f8(_qpack(np.ascontiguousarray(w1[e].T)) * WS),
            "w38": _f8(_qpack(np.ascontiguousarray(w3[e].T)) * W3S),
            "w28": _f8(np.ascontiguousarray(
                w2[e].T.reshape(I // 256, 2, 128, H).transpose(0, 2, 1, 3)) * WS),
            "web": np.broadcast_to(web, (128, cap)).copy(),
        })
    res2 = run_bass_kernel_spmd(nc2, in_maps2, core_ids=list(range(NCORES)),
                                trace=TRACE)
    LAST_RESULTS.append(res2)

    out = h1.copy()
    for e in range(E):
        tok, _ = idx_lists[e]
        out[tok] += res2.results[e]["yT"][:, :len(tok)].T.astype(np.float32)

    return out.reshape(B, S, H).astype(np.float32)


# revision 28
# speedup vs baseline: 2.0150x; 1.0360x over previous
"""Trainium2 Bass kernel for a Mixtral decoder layer (attention + top-2 MoE).

Strategy (8 NeuronCores), all matmuls fp8-e4m3 with DoubleRow perf mode
(2 K-tiles contracted per instruction at 0.5 cycles/row):

  Launch 1 (attention): 2D shard = (batch b in {0,1}) x (head-group g in
    {0..3}, 4 heads / 256 feature slice each). Q/K projected into a
    [h*32+d%32, d//32, t] layout so transposed-score matmuls run as 32-row
    DoubleRow tiles (tile_position=(32h,0)). Softmax exp is split across
    three engines: ACT (exact exp, scale+bias so probs land in fp8 range)
    and DVE->Pool (Schraudolph fast-exp: int32 affine + bitcast convert).
    AV contracts 2 key-tiles per DR matmul with a ones-column folding the
    denominator; output projection on-device, host sums partials.
  Host: residual add, rmsnorm, gating logits, exact top-2 routing,
    per-expert token gather (expert-parallel dispatch in numpy).
  Launch 2 (MoE FFN): expert-parallel - core e owns expert e, processes its
    routed tokens (padded to capacity) with DR-fp8 w1/w3/w2 matmuls; silu
    via ACT (scale folds the weight upscale), gate multiply on DVE.
  Weights are upscaled (x64 / x16) so fp8-e4m3 avoids subnormals; scales
  are folded into activation scale args, psum-copy scales and host-side
  routing weights. fp32 PSUM accumulation throughout.
"""
import os
import sys

import numpy as np
import ml_dtypes

for _p in ("/root/.axon_site", "/root/.axon_site/_ro/trn_rl_repo", "/opt/trn_rl_repo"):
    if os.path.isdir(_p) and _p not in sys.path:
        sys.path.append(_p)

import concourse.tile as tile
from concourse import bacc, mybir
from concourse.bass_utils import run_bass_kernel_spmd

BF16 = ml_dtypes.bfloat16
F8 = ml_dtypes.float8_e4m3
AF = mybir.ActivationFunctionType
ALU = mybir.AluOpType
DT = mybir.dt
PM = mybir.MatmulPerfMode

H = 1024
S = 2048
B = 2
NH = 16
D = 64
E = 8
I = 2048
T = B * S
EPS = 1e-5

NCORES = 8
NGRP = 4              # head groups (cores per batch)
NHPC = NH // NGRP     # 4 heads per core
DS = NHPC * D         # 256-wide feature slice per core
NTK = S // 128        # 16 tk tiles
NQ = H // 256         # 4 DR contraction chunks (pairs of 128)

C = 1152              # MoE expert token capacity (per-expert max on this data ~1087)

WS = 64.0             # weight upscale for wq/wk/wv/wo/w1/w2
W3S = 16.0            # upscale for w3 (keeps hh = 16*silu*g under fp8 max 240)
AOS = 32.0            # attention-out staging scale (fp8 subnormal dodge)
LN16 = 1.3862943611198906     # ln(4): exp bias; probs*4 max ~60 << fp8 max 240
EXPA = 1512775.3951951858     # 0.125 * 2^23 / ln2  (fast-exp scale)
EXPB = 1081643893.0           # (129 - 0.058) * 2^23 (fast-exp offset)

# exp column split per score tile [128,1024]: ACT takes [0:EXPQ), DVE+Pool rest
EXPQ = 672
# h1p eviction split (16 tq tiles): which go to ACT (rest DVE)
EVICT_ACT = frozenset((0, 2, 4, 6, 8, 10, 12, 14))

_CACHE = {}
LAST_RESULTS = []     # BassKernelResults of the last kernel() call (for test harness)
TRACE = os.environ.get("KERNEL_TRACE", "0") == "1"


def _capacity_chunks(cap):
    out, o = [], 0
    while o < cap:
        ln = min(512, cap - o)
        out.append((o, ln))
        o += ln
    return out


def _build_l1():
    nc = bacc.Bacc("TRN2", target_bir_lowering=False, debug=False, num_devices=NCORES)
    x8 = nc.dram_tensor("x8", [NQ, 128, 2, S], DT.float8e4, kind="ExternalInput")
    wq8 = nc.dram_tensor("wq8", [NQ, 128, 2, 2, 128], DT.float8e4, kind="ExternalInput")
    wk8 = nc.dram_tensor("wk8", [NQ, 128, 2, 2, 128], DT.float8e4, kind="ExternalInput")
    wv8 = nc.dram_tensor("wv8", [NQ, 128, 2, DS], DT.float8e4, kind="ExternalInput")
    wo8 = nc.dram_tensor("wo8", [128, 2, H], DT.float8e4, kind="ExternalInput")
    h1p = nc.dram_tensor("h1p", [S, H], DT.bfloat16, kind="ExternalOutput")

    with tile.TileContext(nc) as tc:
        with tc.tile_pool(name="wpool", bufs=1) as wpool, \
             tc.tile_pool(name="qk", bufs=1) as qkpool, \
             tc.tile_pool(name="pt", bufs=6) as ptpool, \
             tc.tile_pool(name="i32", bufs=3) as i32pool, \
             tc.tile_pool(name="rc", bufs=4) as rcpool, \
             tc.tile_pool(name="hout", bufs=3) as hpool, \
             tc.tile_pool(name="dram", bufs=2, space="DRAM") as drpool, \
             tc.tile_pool(name="pp", bufs=3, space="PSUM") as pp, \
             tc.tile_pool(name="pav", bufs=2, space="PSUM") as pav:

            # ---- input loads, ordered by first use ----
            xts = [wpool.tile([128, 2, S], DT.float8e4, name=f"xt{q}", tag=f"xt{q}")
                   for q in range(NQ)]
            nc.sync.dma_start(xts[0][:], x8[0])
            wq_sb = wpool.tile([128, NQ, 2, 2, 128], DT.float8e4)
            nc.sync.dma_start(wq_sb[:], wq8.rearrange("q p j a m -> p q j a m"))
            wk_sb = wpool.tile([128, NQ, 2, 2, 128], DT.float8e4)
            nc.sync.dma_start(wk_sb[:], wk8.rearrange("q p j a m -> p q j a m"))
            for q in range(1, NQ):
                nc.sync.dma_start(xts[q][:], x8[q])
            wv_sb = wpool.tile([128, NQ, 2, DS], DT.float8e4)
            nc.sync.dma_start(wv_sb[:], wv8.rearrange("q p j m -> p q j m"))
            wo_sb = wpool.tile([128, 2, H], DT.float8e4)
            nc.sync.dma_start(wo_sb[:], wo8[:, :, :])
            biast = wpool.tile([128, 1], DT.float32)
            nc.vector.memset(biast[:], LN16)
            bfast = wpool.tile([128, 1], DT.float32)
            nc.vector.memset(bfast[:], EXPB)
            ones64 = wpool.tile([1, 64], DT.bfloat16)
            nc.vector.memset(ones64[:], 1.0)

            # PE warmup: dummy matmuls on memset tiles during the DMA wait
            # so the projection matmuls run at full clock (HAM ramp).
            wa = wpool.tile([128, 64], DT.bfloat16)
            nc.vector.memset(wa[:], 0.0)
            wb = wpool.tile([128, 512], DT.bfloat16)
            nc.vector.memset(wb[:], 0.0)
            wp = pp.tile([128, 512], DT.float32, tag="pp", name="warm")
            for _ in range(8):
                nc.tensor.matmul(wp[0:64, :], wa[:], wb[:],
                                 start=True, stop=True)
            wsink = wpool.tile([64, 64], DT.bfloat16)
            nc.vector.tensor_copy(wsink[:], wp[0:64, 0:64])

            # qt/kt layout: [h*32+d32, jq(d-half), t]
            qt = qkpool.tile([128, 2, S], DT.float8e4, name="qt", tag="qt")
            kt = qkpool.tile([128, 2, S], DT.float8e4, name="kt", tag="kt")
            # vt layout: [tk%128, r(tk pair), j(tk in pair), h*68 + (64 dims,
            # ones, pad)] - head stride 68 keeps the DR pair-dim stride
            # (2*4*68=272 B) a multiple of 16 (s3_lw_dual_fp8_restrictions)
            vt = qkpool.tile([128, NTK // 2, 2, 4 * 68], DT.float8e4,
                             name="vt", tag="vt")
            # aoT layout: [ds%128, ds//128, t] (ds = h*64+d feature slice)
            aoT = qkpool.tile([128, 2, S], DT.float8e4, name="aoT", tag="aoT")

            def qk_proj_unit(wsb, dst, jq, tqh, eng="dve"):
                # dst[h*32+d32, jq, t] = (z1 @ w.T)[g*256 + h*64 + jq*32+d32, t]
                ps = pp.tile([128, 1024], DT.float32, tag="pp", name="ps")
                for half in range(2):
                    sl = slice((2 * tqh + half) * 512,
                               (2 * tqh + half + 1) * 512)
                    for q in range(NQ):
                        nc.tensor.matmul(
                            ps[:, half * 512:(half + 1) * 512],
                            wsb[:, q, :, jq, :],
                            xts[q][:, :, sl],
                            start=(q == 0), stop=(q == NQ - 1),
                            perf_mode=PM.DoubleRow,
                        )
                d = dst[:, jq, tqh * 1024:(tqh + 1) * 1024]
                if eng == "act":
                    nc.scalar.activation(d, ps[:], AF.Copy, scale=1.0 / WS)
                else:
                    nc.vector.tensor_scalar_mul(d, ps[:], 1.0 / WS)

            def v_init():
                # ones columns (and zero pad)
                nc.vector.memset(vt[:, :, :, :].rearrange(
                    "p r j (h c) -> p (r j h) c", c=68)[:, :, 64:68], 0.0)
                nc.vector.memset(vt[:, :, :, :].rearrange(
                    "p r j (h c) -> p (r j h) c", c=68)[:, :, 64:65], 1.0)

            def v_group(grp):
                # 4 tk tiles per psum tile
                pv = pp.tile([128, 1024], DT.float32, tag="pp", name="pv")
                for i4 in range(4):
                    tkt = grp * 4 + i4
                    for q in range(NQ):
                        nc.tensor.matmul(
                            pv[:, i4 * 256:i4 * 256 + DS],
                            xts[q][:, :, tkt * 128:(tkt + 1) * 128],
                            wv_sb[:, q, :, :],
                            start=(q == 0), stop=(q == NQ - 1),
                            perf_mode=PM.DoubleRow,
                        )
                # pv cols (tkt4, h, 64) -> vt[:, r0+{0,1}, j, h*68:+64]
                r0 = grp * 2
                dst = vt[:, r0:r0 + 2, :, :].rearrange(
                    "p r j (h c) -> p (r j h) c", c=68)[:, :, 0:64]
                nc.vector.tensor_scalar_mul(
                    dst, pv[:].rearrange("p (a c) -> p a c", c=64), 1.0 / WS)

            def core_open(h):
                return [pav.tile([65, 512], DT.float32, tag="pav",
                                 name=f"av{h}_{i}") for i in range(2)]

            def av_mms(h, av, ptt, r):
                for i in range(2):
                    nc.tensor.matmul(
                        av[i][:],
                        vt[:, r, :, h * 68:h * 68 + 65],
                        ptt[:, :, i * 512:(i + 1) * 512],
                        start=(r == 0), stop=(r == NTK // 2 - 1),
                        perf_mode=PM.DoubleRow,
                    )

            def core_step(h, tqh, r, av):
                # scores + exp for tk pair r of head h (one tq half); the AV
                # matmuls for the PREVIOUS pair are emitted after this pair's
                # scores so PE never head-of-line blocks on exp latency.
                # exp is column-split: ACT handles [0:EXPQ) (exact exp),
                # DVE->Pool handles [EXPQ:1024) (Schraudolph fast-exp).
                hb = h * 32
                ptt = ptpool.tile([128, 2, 1024], DT.float8e4, tag="pt")
                for j in range(2):
                    tkt = 2 * r + j
                    sc = pp.tile([128, 1024], DT.float32, tag="pp", name="sc")
                    for i in range(2):
                        sl = slice((2 * tqh + i) * 512, (2 * tqh + i + 1) * 512)
                        nc.tensor.matmul(
                            sc[:, i * 512:(i + 1) * 512],
                            kt[hb:hb + 32, :, tkt * 128:(tkt + 1) * 128],
                            qt[hb:hb + 32, :, sl],
                            start=True, stop=True,
                            perf_mode=PM.DoubleRow,
                            tile_position=(hb, 0),
                        )
                    nc.scalar.activation(
                        ptt[:, j, 0:EXPQ], sc[:, 0:EXPQ], AF.Exp,
                        scale=0.125, bias=biast[:])
                    it = i32pool.tile([128, 1024 - EXPQ], DT.int32, tag="i32")
                    nc.vector.scalar_tensor_tensor(
                        it[:], sc[:, EXPQ:1024], EXPA,
                        bfast[:].to_broadcast([128, 1024 - EXPQ]),
                        ALU.mult, ALU.add)
                    nc.gpsimd.tensor_copy(
                        ptt[:, j, EXPQ:1024], it[:].bitcast(DT.float32))
                return (ptt, r)

            def norm_recip(av):
                # rb = 1/denom broadcast to 64 partitions via a DMA round-trip
                # (off-engine; latency is hidden by the deferred apply)
                rc = rcpool.tile([1, 1024], DT.bfloat16, tag="rc")
                with nc.allow_low_precision(reason="softmax denom recip, bf16"):
                    for i in range(2):
                        nc.vector.reciprocal(rc[0:1, i * 512:(i + 1) * 512],
                                             av[i][64:65, :])
                rd = drpool.tile([1, 1024], DT.bfloat16)
                nc.sync.dma_start(rd[:], rc[:])
                rb = rcpool.tile([64, 1024], DT.bfloat16, tag="rb")
                nc.sync.dma_start(rb[:], rd[:].to_broadcast([64, 1024]))
                return rb

            def norm_apply(h, tqh, av, rb):
                ro = (h % 2) * 64
                for i in range(2):
                    sl = slice((2 * tqh + i) * 512, (2 * tqh + i + 1) * 512)
                    nc.vector.scalar_tensor_tensor(
                        aoT[ro:ro + 64, h // 2, sl],
                        av[i][0:64, :], AOS, rb[:, i * 512:(i + 1) * 512],
                        ALU.mult, ALU.mult)

            def norm_final(h, tqh, av):
                # last attend: no DMA round-trip on the critical tail. recip
                # -> K=1 matmul broadcast into PSUM -> two-step apply (DVE
                # can read at most one PSUM operand per instruction).
                rc = rcpool.tile([1, 1024], DT.bfloat16, tag="rc")
                with nc.allow_low_precision(reason="softmax denom recip, bf16"):
                    for i in range(2):
                        nc.vector.reciprocal(rc[0:1, i * 512:(i + 1) * 512],
                                             av[i][64:65, :])
                rbp = pp.tile([64, 1024], DT.float32, tag="pp", name="rbp")
                for i in range(2):
                    nc.tensor.matmul(
                        rbp[:, i * 512:(i + 1) * 512],
                        ones64[0:1, :], rc[0:1, i * 512:(i + 1) * 512],
                        start=True, stop=True)
                ro = (h % 2) * 64
                tmp = rcpool.tile([64, 1024], DT.bfloat16, tag="rb")
                for i in range(2):
                    nc.vector.tensor_scalar_mul(
                        tmp[:, i * 512:(i + 1) * 512], av[i][0:64, :], AOS)
                for i in range(2):
                    sl = slice((2 * tqh + i) * 512, (2 * tqh + i + 1) * 512)
                    nc.vector.tensor_tensor(
                        aoT[ro:ro + 64, h // 2, sl],
                        tmp[:, i * 512:(i + 1) * 512],
                        rbp[:, i * 512:(i + 1) * 512], ALU.mult)

            def o_proj(oc):
                # h1p rows oc*128..: psum = 2048 * true value (host rescales)
                po = pp.tile([128, 1024], DT.float32, tag="pp", name="po")
                for jc in range(2):
                    nc.tensor.matmul(
                        po[:, jc * 512:(jc + 1) * 512],
                        aoT[:, :, oc * 128:(oc + 1) * 128],
                        wo_sb[:, :, jc * 512:(jc + 1) * 512],
                        start=True, stop=True,
                        perf_mode=PM.DoubleRow,
                    )
                ht = hpool.tile([128, H], DT.bfloat16, tag="ht")
                nc.scalar.activation(ht[:, 0:512], po[:, 0:512], AF.Copy)
                nc.vector.tensor_copy(ht[:, 512:1024], po[:, 512:1024])
                nc.sync.dma_start(h1p[oc * 128:(oc + 1) * 128, :], ht[:])

            def attend(h, tqh, prev, inject=None, last=False):
                # prev = (h', tqh', av', rc') of the previous attend; its
                # broadcast+apply is emitted after step 0 so the PE FIFO never
                # blocks on the reciprocal, and the av-buffer WAR for this
                # attend's lagged AV (emitted from step 1) stays safe.
                av = core_open(h)
                pends = []
                for r in range(NTK // 2):
                    pends.append(core_step(h, tqh, r, av))
                    if len(pends) > 1:
                        av_mms(h, av, *pends.pop(0))
                    if r == 0 and prev is not None:
                        norm_apply(*prev)
                    if inject and r in inject:
                        inject[r]()
                for pe_ in pends:
                    av_mms(h, av, *pe_)
                if last:
                    return av
                return (h, tqh, av, norm_recip(av))

            # ---- schedule ----
            v_init()
            qk_proj_unit(wq_sb, qt, 0, 0, "dve")
            qk_proj_unit(wq_sb, qt, 1, 0, "act")
            qk_proj_unit(wk_sb, kt, 0, 0, "dve")
            qk_proj_unit(wk_sb, kt, 1, 0, "act")

            for g_ in range(4):
                v_group(g_)
            prev = attend(0, 0, None)
            qk_proj_unit(wq_sb, qt, 0, 1, "act")
            qk_proj_unit(wq_sb, qt, 1, 1, "dve")
            prev = attend(1, 0, prev)
            qk_proj_unit(wk_sb, kt, 0, 1, "act")
            qk_proj_unit(wk_sb, kt, 1, 1, "dve")
            prev = attend(2, 0, prev)
            prev = attend(3, 0, prev)
            prev = attend(0, 1, prev)
            prev = attend(1, 1, prev,
                          inject={2: lambda: o_proj(0), 5: lambda: o_proj(1)})
            prev = attend(2, 1, prev,
                          inject={2: lambda: o_proj(2), 4: lambda: o_proj(3),
                                  6: lambda: o_proj(4)})
            prev = attend(3, 1, prev,
                          inject={2: lambda: o_proj(5), 4: lambda: o_proj(6),
                                  6: lambda: o_proj(7)})
            norm_apply(*prev)
            for oc in range(8, 16):
                o_proj(oc)

    nc.compile()
    nc.finalize()
    return nc


def _build_l2(cap):
    nc = bacc.Bacc("TRN2", target_bir_lowering=False, debug=False, num_devices=NCORES)
    ze8 = nc.dram_tensor("ze8", [NQ, 128, 2, cap], DT.float8e4, kind="ExternalInput")
    w18 = nc.dram_tensor("w18", [NQ, 128, 2, I], DT.float8e4, kind="ExternalInput")
    w38 = nc.dram_tensor("w38", [NQ, 128, 2, I], DT.float8e4, kind="ExternalInput")
    w28 = nc.dram_tensor("w28", [I // 256, 128, 2, H], DT.float8e4,
                         kind="ExternalInput")
    web = nc.dram_tensor("web", [128, cap], DT.float32, kind="ExternalInput")
    yT = nc.dram_tensor("yT", [H, cap], DT.bfloat16, kind="ExternalOutput")

    cch = _capacity_chunks(cap)
    NIC = I // 128
    NR = I // 256
    with tile.TileContext(nc) as tc:
        with tc.tile_pool(name="wpool", bufs=1) as wpool, \
             tc.tile_pool(name="hh", bufs=1) as hhpool, \
             tc.tile_pool(name="hs", bufs=3) as hspool, \
             tc.tile_pool(name="yt", bufs=3) as ytpool, \
             tc.tile_pool(name="pg", bufs=8, space="PSUM") as pg:
            py = pg

            zcs = [wpool.tile([128, 2, cap], DT.float8e4, name=f"zc{q}",
                              tag=f"zc{q}") for q in range(NQ)]
            w1cs = [wpool.tile([128, 2, I], DT.float8e4, name=f"w1c{q}",
                               tag=f"w1c{q}") for q in range(NQ)]
            w3cs = [wpool.tile([128, 2, I], DT.float8e4, name=f"w3c{q}",
                               tag=f"w3c{q}") for q in range(NQ)]
            zh = cap // 2
            for q in range(NQ):
                nc.sync.dma_start(zcs[q][:, :, 0:zh], ze8[q][:, :, 0:zh])
                nc.sync.dma_start(w1cs[q][:, :, 0:256], w18[q][:, :, 0:256])
            for q in range(NQ):
                nc.sync.dma_start(zcs[q][:, :, zh:cap], ze8[q][:, :, zh:cap])
                nc.sync.dma_start(w1cs[q][:, :, 256:I], w18[q][:, :, 256:I])
            for q in range(NQ):
                nc.sync.dma_start(w3cs[q][:, :, 0:256], w38[q][:, :, 0:256])
                nc.sync.dma_start(w3cs[q][:, :, 256:I], w38[q][:, :, 256:I])
            web_sb = wpool.tile([128, cap], DT.float32)
            nc.sync.dma_start(web_sb[:], web[:, :])

            # hh[i%128, i//256, (i//128)%2, c] fp8, = 16*silu(z@w1)*(z@w3)
            hhs = [hhpool.tile([128, 2, cap], DT.float8e4, name=f"hh{r}",
                               tag=f"hh{r}") for r in range(NR)]
            w2_holder = []

            for ic in range(NIC):
                hp = [pg.tile([128, 512], DT.float32, tag="pg", name=f"hp{j}")
                      for j in range(len(cch))]
                for q in range(NQ):
                    for j, (o, ln) in enumerate(cch):
                        nc.tensor.matmul(
                            hp[j][:, 0:ln],
                            w1cs[q][:, :, ic * 128:(ic + 1) * 128],
                            zcs[q][:, :, o:o + ln],
                            start=(q == 0), stop=(q == NQ - 1),
                            perf_mode=PM.DoubleRow,
                        )
                hs = hspool.tile([128, cap], DT.bfloat16, tag="hs", name="hs")
                for j, (o, ln) in enumerate(cch):
                    nc.scalar.activation(hs[:, o:o + ln], hp[j][:, 0:ln],
                                         AF.Silu, scale=1.0 / WS)
                gp = [pg.tile([128, 512], DT.float32, tag="pg", name=f"gp{j}")
                      for j in range(len(cch))]
                for q in range(NQ):
                    for j, (o, ln) in enumerate(cch):
                        nc.tensor.matmul(
                            gp[j][:, 0:ln],
                            w3cs[q][:, :, ic * 128:(ic + 1) * 128],
                            zcs[q][:, :, o:o + ln],
                            start=(q == 0), stop=(q == NQ - 1),
                            perf_mode=PM.DoubleRow,
                        )
                for j, (o, ln) in enumerate(cch):
                    nc.vector.tensor_tensor(
                        hhs[ic // 2][:, ic % 2, o:o + ln],
                        gp[j][:, 0:ln], hs[:, o:o + ln], ALU.mult)
                if ic == 0:
                    # emit w2 load after the first h-block for DMA priority
                    w2_sb = wpool.tile([128, NR, 2, H], DT.float8e4)
                    nc.sync.dma_start(w2_sb[:], w28.rearrange("r p j m -> p r j m"))
                    w2_holder.append(w2_sb)

            w2_sb = w2_holder[0]
            for hc in range(H // 128):
                yt = ytpool.tile([128, cap], DT.bfloat16, tag="yt", name="yt")
                for j, (o, ln) in enumerate(cch):
                    yp = py.tile([128, 512], DT.float32, tag="pg", name="yp")
                    for r in range(NR):
                        nc.tensor.matmul(
                            yp[:, 0:ln],
                            w2_sb[:, r, :, hc * 128:(hc + 1) * 128],
                            hhs[r][:, :, o:o + ln],
                            start=(r == 0), stop=(r == NR - 1),
                            perf_mode=PM.DoubleRow,
                        )
                    nc.vector.tensor_tensor(
                        yt[:, o:o + ln], yp[:, 0:ln], web_sb[:, o:o + ln],
                        ALU.mult)
                nc.sync.dma_start(yT[hc * 128:(hc + 1) * 128, :], yt[:])

    nc.compile()
    nc.finalize()
    return nc


def _get(name, builder, *args):
    if name not in _CACHE:
        _CACHE[name] = builder(*args)
    return _CACHE[name]


def _rmsnorm(x, w):
    xf = x.astype(np.float32)
    rms = 1.0 / np.sqrt((xf * xf).mean(axis=-1, keepdims=True) + EPS)
    return (xf * rms) * w.astype(np.float32)


def _f8(a):
    return np.clip(a, -240.0, 240.0).astype(F8)


def _qpack(mat_T):
    """[H, M] -> [H//256, 128, 2, M] with row (q*2+j)*128+p -> [q, p, j, :]."""
    M = mat_T.shape[1]
    return np.ascontiguousarray(
        mat_T.reshape(NQ, 2, 128, M).transpose(0, 2, 1, 3))


def kernel(x, ln1_w, ln2_w, wq, wk, wv, wo, gate_w, w1, w2, w3):
    global LAST_RESULTS
    LAST_RESULTS = []
    x = np.asarray(x, np.float32)
    wq, wk, wv, wo = (np.asarray(a, np.float32) for a in (wq, wk, wv, wo))
    gate_w = np.asarray(gate_w, np.float32)
    w1, w2, w3 = (np.asarray(a, np.float32) for a in (w1, w2, w3))
    ln1_w = np.asarray(ln1_w, np.float32)
    ln2_w = np.asarray(ln2_w, np.float32)

    xf = x.reshape(T, H)
    z1 = _rmsnorm(xf, ln1_w)
    # ---- launch 1: attention ----
    nc1 = _get("l1", _build_l1)
    in_maps = []
    for c in range(NCORES):
        b, g = divmod(c, NGRP)
        x8 = _f8(_qpack(np.ascontiguousarray(z1[b * S:(b + 1) * S].T)))
        # wq8[q, p, jk, jq, h*32+d] = wq[g*256 + h*64 + jq*32 + d, (q*2+jk)*128+p]
        wqg = wq[g * DS:(g + 1) * DS] * WS   # [256, H]
        wkg = wk[g * DS:(g + 1) * DS] * WS
        wvg = wv[g * DS:(g + 1) * DS] * WS
        wog = wo[:, g * DS:(g + 1) * DS] * WS  # [H, 256]

        def _qk_pack(w):
            a = _qpack(np.ascontiguousarray(w.T))       # [q, p, jk, 256]
            a = a.reshape(NQ, 128, 2, NHPC, 2, 32)       # f = h*64+jq*32+d
            return _f8(np.ascontiguousarray(
                a.transpose(0, 1, 2, 4, 3, 5).reshape(NQ, 128, 2, 2, 128)))

        wo8 = np.ascontiguousarray(
            wog.T.reshape(2, 128, H).transpose(1, 0, 2))  # [p, j, H]
        in_maps.append({
            "x8": x8,
            "wq8": _qk_pack(wqg),
            "wk8": _qk_pack(wkg),
            "wv8": _f8(_qpack(np.ascontiguousarray(wvg.T))),
            "wo8": _f8(wo8),
        })
    res1 = run_bass_kernel_spmd(nc1, in_maps, core_ids=list(range(NCORES)),
                                trace=TRACE)
    LAST_RESULTS.append(res1)

    h1 = xf.copy()
    for c in range(NCORES):
        b = c // NGRP
        h1[b * S:(b + 1) * S] += res1.results[c]["h1p"].astype(np.float32) \
            / (AOS * WS)

    # ---- host: routing (exact fp32 semantics like the reference) ----
    z = _rmsnorm(h1, ln2_w)
    logits = (z.astype(np.float64) @ gate_w.T.astype(np.float64)).astype(np.float32)
    order = np.argsort(-logits, axis=-1, kind="stable")
    sel = order[:, :2]                               # top-2, ties -> lower index
    vals = np.take_along_axis(logits, sel, axis=-1).astype(np.float32)
    mx = vals.max(axis=-1, keepdims=True)
    ex = np.exp(vals - mx)
    rw = (ex / ex.sum(axis=-1, keepdims=True)).astype(np.float32)

    idx_lists = []
    for e in range(E):
        m = (sel == e)
        tok = np.nonzero(m.any(axis=-1))[0]
        wgt = np.where(m, rw, 0.0).sum(axis=-1)[tok]
        idx_lists.append((tok, wgt.astype(np.float32)))
    maxload = max(len(tok) for tok, _ in idx_lists)
    cap = C
    while cap < maxload:
        cap += 512
    nc2 = _get(f"l2_{cap}", _build_l2, cap)

    # ---- launch 2: expert-parallel FFN ----
    zT = np.ascontiguousarray(z.T)                   # [H, T] fp32
    in_maps2 = []
    for e in range(E):
        tok, wgt = idx_lists[e]
        zeT = np.zeros((H, cap), np.float32)
        zeT[:, :len(tok)] = zT[:, tok]
        web = np.zeros((cap,), np.float32)
        web[:len(tok)] = wgt / (WS * W3S)
        in_maps2.append({
            "ze8": _f8(_qpack(zeT)),
            "w18": _f8(_qpack(np.ascontiguousarray(w1[e].T)) * WS),
            "w38": _f8(_qpack(np.ascontiguousarray(w3[e].T)) * W3S),
            "w28": _f8(np.ascontiguousarray(
                w2[e].T.reshape(I // 256, 2, 128, H).transpose(0, 2, 1, 3)) * WS),
            "web": np.broadcast_to(web, (128, cap)).copy(),
        })
    res2 = run_bass_kernel_spmd(nc2, in_maps2, core_ids=list(range(NCORES)),
                                trace=TRACE)
    LAST_RESULTS.append(res2)

    out = h1.copy()
    for e in range(E):
        tok, _ = idx_lists[e]
        out[tok] += res2.results[e]["yT"][:, :len(tok)].T.astype(np.float32)

    return out.reshape(B, S, H).astype(np.float32)
